# revision 9
# baseline (speedup 1.0000x reference)
"""Trainium2 Bass kernel for nn_DotAttention (B=8 data-parallel over 8 cores).

Per core (one batch element), bf16 with one fp8 DoubleRow stage. v3:
the ACT engine runs the 32 exps back-to-back with no table switches in
between (RELU projections on DVE, sigmoids deferred behind a fake late
dependency); the PE round-robins [next-pair scores, U(t), one small
background quantum] per exp pair, staying dense to hold its p-state.

  x.T/m.T   : fp32 PE transposes; PSUM->SBUF copy casts to bf16 (DVE)
  xp/mp     : W.T @ {x,m}.T into PSUM per 256 cols; bias+relu via DVE
  S.T       : mp.T(:,jtile) @ xp.T, bf16, K=96
  e8        : exp(S.T*scale + maskbias) -> fp8e4 on ACT (table 0 only)
  U[jx,151] : fp8 DoubleRow vs [m|1]; denominator in col 150; stride 171
  normalize : reciprocal_approx_fast + per-partition tensor_scalar -> bf16
  gate      : res.T chunks stationary, Wg moving; logits copied to SBUF
  tail      : one table switch, sigmoid per 2-chunk pair (bias tied to the
              last exp so the scheduler cannot hoist it), gate*res mults,
              output DMA per pair alternating sync/gpsimd queues
DMA: sync m0..m3+mask+Wg+bg, scalar(ACT) x0..x3, gpsimd Wi/bi/Wm/bm.
PSUM: scores 2x2 banks, U 3 (stride 171), shared small bank, tail gates
reuse the freed score banks.
"""

import contextlib
import math

import numpy as np

import concourse.bass as bass
import concourse.mybir as mybir
import concourse.tile as tile
from concourse import bacc
from concourse.bass_utils import run_bass_kernel_spmd
from concourse.masks import make_identity

F32 = mybir.dt.float32
F16 = mybir.dt.bfloat16
F8 = mybir.dt.float8e4
I32 = mybir.dt.int32
DR = mybir.MatmulPerfMode.DoubleRow

B = 8
JX = 2048
JM = 2048
D = 150
H = 96
G = 300
NJT = 16          # jm tiles of 128
NCH = 16          # jx chunks of 128
HALF = 1024
NSUB = HALF // 512
SCALE = 1.0 / math.sqrt(float(H))
NEG_BIG = 1.0e30


def _body(tc, x_d, m_d, mask_d, wi_d, bi_d, wm_d, bm_d, wg_d, bg_d, o_d):
    nc = tc.nc
    Exp = mybir.ActivationFunctionType.Exp
    Sigmoid = mybir.ActivationFunctionType.Sigmoid
    MUL = mybir.AluOpType.mult
    SUB = mybir.AluOpType.subtract
    ADD = mybir.AluOpType.add
    MAX = mybir.AluOpType.max

    with contextlib.ExitStack() as ctx:
        const = ctx.enter_context(tc.tile_pool(name="const", bufs=1))
        work = ctx.enter_context(tc.tile_pool(name="work", bufs=2))
        epool = ctx.enter_context(tc.tile_pool(name="epool", bufs=3))
        psb = ctx.enter_context(tc.tile_pool(name="psb", bufs=2, space="PSUM"))
        pu = ctx.enter_context(tc.tile_pool(name="pu", bufs=1, space="PSUM"))

        # ---- gpsimd queue head: identities -------------------------------
        ident16 = const.tile([128, 128], F16)
        make_identity(nc, ident16)
        ident32s = const.tile([NJT, NJT], F32)
        make_identity(nc, ident32s)
        ident32 = const.tile([128, 128], F32)
        make_identity(nc, ident32)

        # scalar queue head: exp table preload (table_sel=0)
        dummy = const.tile([1, 1], F32)
        nc.scalar.activation(out=dummy, in_=ident16[0:1, 0:1], func=Exp, scale=1.0)

        # ---- input DMAs ---------------------------------------------------
        # sync: m groups first, then mask, then late-needed Wg/bg.
        # scalar(ACT): x groups (idle until the first exp).
        # gpsimd: Wi/bi/Wm/bm for the projections.
        x_nat = const.tile([128, NCH, D], F32)
        m_nat = const.tile([128, NJT, D], F32)
        x_re = x_d.rearrange("(n p) d -> p n d", p=128)
        m_re = m_d.rearrange("(n p) d -> p n d", p=128)
        for g in range(4):
            gs4 = slice(g * 4, (g + 1) * 4)
            nc.sync.dma_start(out=m_nat[:, gs4, :], in_=m_re[:, gs4, :])
            nc.scalar.dma_start(out=x_nat[:, gs4, :], in_=x_re[:, gs4, :])
        mask_sb = const.tile([NJT, 128], I32)
        nc.sync.dma_start(out=mask_sb, in_=mask_d.rearrange("(n p) -> n p", p=128))

        wstage = const.tile([128, 2 * H], F32)
        wstage2 = const.tile([D - 128, 2 * H], F32)
        bi_sb = const.tile([H, 1], F32)
        bm_sb = const.tile([H, 1], F32)
        nc.gpsimd.dma_start(out=wstage[:, 0:H], in_=wi_d[0:128, :])
        nc.gpsimd.dma_start(out=wstage2[:, 0:H], in_=wi_d[128:D, :])
        nc.gpsimd.dma_start(out=bi_sb, in_=bi_d.rearrange("(n one) -> n one", one=1))
        nc.gpsimd.dma_start(out=wstage[:, H : 2 * H], in_=wm_d[0:128, :])
        nc.gpsimd.dma_start(out=wstage2[:, H : 2 * H], in_=wm_d[128:D, :])
        nc.gpsimd.dma_start(out=bm_sb, in_=bm_d.rearrange("(n one) -> n one", one=1))

        # ---- PE warmup while the first DMAs land -------------------------
        jp = psb.tile([128, 128], F32, tag="sm", name="junk", bufs=1)
        for _ in range(12):
            nc.tensor.matmul(
                jp, ident16, ident16, start=True, stop=True,
                skip_group_check=True)
        nc.vector.tensor_copy(out=dummy, in_=jp[0:1, 0:1])

        # ---- weight casts to bf16 (vector, tiny) -------------------------
        wi16a = const.tile([128, H], F16)
        nc.vector.tensor_copy(out=wi16a, in_=wstage[:, 0:H])
        wi16b = const.tile([D - 128, H], F16)
        nc.vector.tensor_copy(out=wi16b, in_=wstage2[:, 0:H])
        wm16a = const.tile([128, H], F16)
        nc.vector.tensor_copy(out=wm16a, in_=wstage[:, H : 2 * H])
        wm16b = const.tile([D - 128, H], F16)
        nc.vector.tensor_copy(out=wm16b, in_=wstage2[:, H : 2 * H])

        # ---- fp8 m (+ones col) for the U matmuls, 2-chunk units ----------
        mt8 = const.tile([128, NJT, 176], F8)
        nc.gpsimd.memset(mt8[:, :, D:176], 0.0)
        nc.gpsimd.memset(mt8[:, :, 150:151], 1.0)
        for u in range(8):
            u2 = slice(u * 2, u * 2 + 2)
            nc.gpsimd.tensor_copy(out=mt8[:, u2, 0:D], in_=m_nat[:, u2, :])

        # ---- mask -> additive exp bias [128, NJT] ------------------------
        maskf = const.tile([NJT, 128], F32)
        nc.vector.tensor_copy(out=maskf, in_=mask_sb)
        nc.vector.tensor_scalar(
            out=maskf, in0=maskf, scalar1=1.0, scalar2=NEG_BIG,
            op0=SUB, op1=MUL)
        mb_ps = psb.tile([128, NJT], F32, tag="sm", name="mbps", bufs=1)
        nc.tensor.transpose(mb_ps, maskf, ident32s)
        maskbias = const.tile([128, NJT], F32)
        nc.vector.tensor_copy(out=maskbias, in_=mb_ps)

        # ---- transposed bf16 layouts --------------------------------------
        xT16a = const.tile([128, JX], F16)
        mT16a = const.tile([128, JM], F16)
        mT16b = const.tile([D - 128, JM], F16)
        # merged tail: x.T tail rows 0..21, U.T tail rows 32..53, ones row 64
        rtail = const.tile([65, JX], F16)
        nc.vector.memset(rtail, 0.0)
        nc.vector.memset(rtail[64:65, :], 1.0)

        xpT16 = const.tile([H, JX], F16)
        mpT16 = const.tile([H, JM], F16)

        def unit(which, p):
            # one 2-chunk (256-col) unit: fp32 transpose piece + cast-copy
            # + 256-col projection + DVE relu
            if which == "x":
                src, dstA, dstB = x_nat, xT16a, rtail
                wa, wb, b_sb, dst = wi16a, wi16b, bi_sb, xpT16
            else:
                src, dstA, dstB = m_nat, mT16a, mT16b
                wa, wb, b_sb, dst = wm16a, wm16b, bm_sb, mpT16
            pT = psb.tile([128, 2, 256], F32, tag="sm", name="pT", bufs=1)
            for i in range(2):
                c = p * 2 + i
                nc.tensor.transpose(pT[:, i, 0:128], src[:, c, 0:128], ident32)
                nc.tensor.transpose(
                    pT[0 : D - 128, i, 128:256], src[:, c, 128:D], ident32)
            ss = slice(p * 256, (p + 1) * 256)
            nc.vector.tensor_copy(out=dstA[:, ss], in_=pT[:, :, 0:128])
            nc.vector.tensor_copy(
                out=dstB[0 : D - 128, ss], in_=pT[0 : D - 128, :, 128:256])
            pp = psb.tile([H, 256], F32, tag="sm", name="pp", bufs=1)
            nc.tensor.matmul(
                pp, wa, dstA[:, ss],
                start=True, stop=False, skip_group_check=True)
            nc.tensor.matmul(
                pp, wb, dstB[0 : D - 128, ss],
                start=False, stop=True, skip_group_check=True)
            nc.vector.tensor_scalar(
                out=dst[:, ss], in0=pp, scalar1=b_sb, scalar2=0.0,
                op0=ADD, op1=MAX)

        state = {"m": 0, "x": 0}

        def need_m(jtiles):
            while state["m"] * 2 < jtiles:
                unit("m", state["m"])
                state["m"] += 1

        def need_x(chunks):
            while state["x"] * 2 < chunks:
                unit("x", state["x"])
                state["x"] += 1

        # ---- Wg/bg staged f32; cast on vector mid-window -----------------
        wg16a = const.tile([128, G], F16, tag="wg16a")
        wg16c = const.tile([128, G], F16, tag="wg16c")
        wgtail = const.tile([65, G], F16, tag="wgtail")
        nc.gpsimd.memset(wgtail, 0.0)
        wg_stages = []
        for sl, (g0, g1), w, r0 in ((0, (0, 128), wg16a, 0),
                                    (1, (128, 150), wgtail, 0),
                                    (2, (150, 278), wg16c, 0),
                                    (3, (278, 300), wgtail, 32)):
            wst = const.tile([g1 - g0, G], F32, tag=f"wgst_{sl}", name=f"wgst{sl}")
            nc.sync.dma_start(out=wst, in_=wg_d[g0:g1, :])
            wg_stages.append((wst, w, r0, g1 - g0))
        bgst = const.tile([1, G], F32, tag="bgst")
        nc.sync.dma_start(out=bgst, in_=bg_d.rearrange("(one n) -> one n", one=1))

        def cast_wg():
            for wst, w, r0, rows in wg_stages:
                nc.vector.tensor_copy(out=w[r0 : r0 + rows, :], in_=wst)
            nc.vector.tensor_copy(out=wgtail[64:65, :], in_=bgst)

        # ---- attention state ----------------------------------------------
        U16n = const.tile([128, NCH, 160], F16)
        nc.vector.memset(U16n[:, :, 150:160], 0.0)
        rcp_all = const.tile([128, NCH], F32)
        uT16a = const.tile([128, JX], F16)
        glog = const.tile([128, NCH, G], F32)
        gate16 = const.tile([128, NCH, G], F16)
        o_re = o_d.rearrange("(n p) k -> p n k", p=128)

        def ut_group(g):
            pA = psb.tile([128, 2, 256], F16, tag="sm", name="pUA", bufs=1)
            for i in range(2):
                c = g * 2 + i
                nc.tensor.transpose(
                    pA[:, i, 0:128], U16n[:, c, 0:128], ident16)
                nc.tensor.transpose(
                    pA[0 : D - 128, i, 128:256], U16n[:, c, 128:D], ident16)
            gcols = slice(g * 256, (g + 1) * 256)
            nc.vector.tensor_copy(out=uT16a[:, gcols], in_=pA[:, :, 0:128])
            nc.vector.tensor_copy(
                out=rtail[32 : 32 + D - 128, gcols],
                in_=pA[0 : D - 128, :, 128:256])

        def gate_chunk(c, tag="sm"):
            cs = slice(c * 128, (c + 1) * 128)
            gp = psb.tile([128, G], F32, tag=tag, name="gp",
                          bufs=1 if tag == "sm" else 2)
            for gi, (lhs, w) in enumerate((
                (xT16a[:, cs], wg16a), (uT16a[:, cs], wg16c),
                (rtail[:, cs], wgtail))):
                nc.tensor.matmul(
                    gp, lhs, w,
                    start=(gi == 0), stop=(gi == 2), skip_group_check=True)
            nc.vector.tensor_copy(out=glog[:, c, :], in_=gp)

        def norm_chunk(c, Up, h):
            nc.vector.tensor_scalar(
                out=U16n[:, c, 0:D], in0=Up[:, c - h * 8, 0:D],
                scalar1=rcp_all[:, c : c + 1],
                scalar2=None, op0=MUL)

        def emit_scores(h, j):
            sp = psb.tile([128, HALF], F32, tag="big", name="sp")
            for sx in range(NSUB):
                ss = slice(h * HALF + sx * 512, h * HALF + (sx + 1) * 512)
                nc.tensor.matmul(
                    sp[:, sx * 512 : (sx + 1) * 512],
                    mpT16[:, j * 128 : (j + 1) * 128], xpT16[:, ss],
                    start=True, stop=True, skip_group_check=True)
            return sp

        # ---- preamble: x chunks 0..7 + m tiles 0..3 -> first scores ------
        need_x(8)
        need_m(4)
        sps = [emit_scores(0, 0), emit_scores(0, 1)]

        # extra background quanta beyond the need_m guards, one per h0 slot
        bg_h0 = [
            lambda: need_m(8),
            lambda: need_x(10),
            lambda: need_x(12),
            lambda: need_m(12),
            lambda: need_x(14),
            lambda: need_x(16),
            lambda: need_m(16),
            lambda: cast_wg(),
        ]

        # ---- attention main loop ------------------------------------------
        Ups = [None, None]
        for h in range(2):
            Up = pu.tile([128, 8, 171], F32, tag="U", name="Up")
            Ups[h] = Up
            e_cur = epool.tile([128, 2, HALF], F8, tag="e8", name="e8")
            for t in range(NJT // 2):
                for s in range(2):
                    j = 2 * t + s
                    nc.scalar.activation(
                        out=e_cur[:, s, :], in_=sps[s], func=Exp,
                        bias=maskbias[:, j : j + 1], scale=SCALE)
                # next-pair scores first: keep the exp stream fed
                if t < NJT // 2 - 1:
                    need_m(2 * t + 4)
                    sps = [emit_scores(h, 2 * t + 2),
                           emit_scores(h, 2 * t + 3)]
                elif h == 0:
                    need_x(16)
                    sps = [emit_scores(1, 0), emit_scores(1, 1)]
                for c in range(8):
                    nc.tensor.matmul(
                        Up[:, c, 0:151],
                        e_cur[:, :, c * 128 : (c + 1) * 128],
                        mt8[:, 2 * t : 2 * t + 2, 0:151],
                        start=(t == 0), stop=(t == NJT // 2 - 1),
                        perf_mode=DR, skip_group_check=True)
                # one light background quantum per slot
                if h == 0:
                    bg_h0[t]()
                else:
                    if t == 0:
                        den = work.tile([128, 8], F32, tag="den")
                        nc.vector.tensor_copy(out=den, in_=Ups[0][:, :, 150])
                        nc.vector.reciprocal_approx_fast(
                            out=rcp_all[:, 0:8], in_=den)
                        for c in range(8):
                            norm_chunk(c, Ups[0], 0)
                        ut_group(0)
                    elif t == 1:
                        gate_chunk(0)
                        gate_chunk(1)
                    elif t == 2:
                        ut_group(1)
                    elif t == 3:
                        gate_chunk(2)
                        gate_chunk(3)
                    elif t == 4:
                        ut_group(2)
                    elif t == 6:
                        ut_group(3)
                if t < NJT // 2 - 1:
                    e_cur = epool.tile([128, 2, HALF], F8, tag="e8", name="e8")

        # ---- tail ---------------------------------------------------------
        # zero bias that depends on the last exp: pins sigmoids after it
        zbias = const.tile([128, 1], F32)
        nc.vector.tensor_scalar(
            out=zbias, in0=e_cur[:, 1, 0:1], scalar1=0.0, scalar2=None,
            op0=MUL)

        # h1 normalize
        den = work.tile([128, 8], F32, tag="den")
        nc.vector.tensor_copy(out=den, in_=Ups[1][:, :, 150])
        nc.vector.reciprocal_approx_fast(out=rcp_all[:, 8:16], in_=den)
        for c in range(8, 16):
            norm_chunk(c, Ups[1], 1)

        def sig_pair(cp):
            c2 = slice(cp * 2, cp * 2 + 2)
            nc.scalar.activation(
                out=gate16[:, c2, :], in_=glog[:, c2, :], func=Sigmoid,
                bias=zbias, scale=1.0)

        def out_pair(cp, dma_eng):
            c2 = slice(cp * 2, cp * 2 + 2)
            onat = work.tile([128, 2, G], F32, tag="onat", bufs=4)
            eng = nc.gpsimd if dma_eng is nc.sync else nc.vector
            eng.tensor_tensor(
                out=onat[:, :, 0:D], in0=gate16[:, c2, 0:D],
                in1=x_nat[:, c2, :], op=MUL)
            eng.tensor_tensor(
                out=onat[:, :, D:G], in0=gate16[:, c2, D:G],
                in1=U16n[:, c2, 0:D], op=MUL)
            dma_eng.dma_start(out=o_re[:, c2, :], in_=onat)

        # gates c4..7 are ready now (uT g2,g3 done in-window); the scores
        # banks are free, so tail gates rotate through the "big" tag for
        # copy/matmul overlap
        gate_chunk(4, tag="big")
        gate_chunk(5, tag="big")
        sig_pair(0)
        out_pair(0, nc.sync)
        gate_chunk(6, tag="big")
        gate_chunk(7, tag="big")
        sig_pair(1)
        out_pair(1, nc.gpsimd)
        ut_group(4)
        gate_chunk(8, tag="big")
        gate_chunk(9, tag="big")
        sig_pair(2)
        out_pair(2, nc.sync)
        ut_group(5)
        gate_chunk(10, tag="big")
        gate_chunk(11, tag="big")
        sig_pair(3)
        out_pair(3, nc.gpsimd)
        ut_group(6)
        gate_chunk(12, tag="big")
        gate_chunk(13, tag="big")
        sig_pair(4)
        out_pair(4, nc.sync)
        ut_group(7)
        gate_chunk(14, tag="big")
        gate_chunk(15, tag="big")
        sig_pair(5)
        out_pair(5, nc.gpsimd)
        sig_pair(6)
        out_pair(6, nc.sync)
        sig_pair(7)
        out_pair(7, nc.gpsimd)


_NC_CACHE = None


def _build_nc():
    global _NC_CACHE
    if _NC_CACHE is not None:
        return _NC_CACHE
    nc = bacc.Bacc(None, target_bir_lowering=False, debug=False)
    x_d = nc.dram_tensor("x", [JX, D], F32, kind="ExternalInput")
    m_d = nc.dram_tensor("m", [JM, D], F32, kind="ExternalInput")
    mask_d = nc.dram_tensor("mask", [JM], I32, kind="ExternalInput")
    wi_d = nc.dram_tensor("Wi", [D, H], F32, kind="ExternalInput")
    bi_d = nc.dram_tensor("bi", [H], F32, kind="ExternalInput")
    wm_d = nc.dram_tensor("Wm", [D, H], F32, kind="ExternalInput")
    bm_d = nc.dram_tensor("bm", [H], F32, kind="ExternalInput")
    wg_d = nc.dram_tensor("Wg", [G, G], F32, kind="ExternalInput")
    bg_d = nc.dram_tensor("bg", [G], F32, kind="ExternalInput")
    o_d = nc.dram_tensor("out", [JX, G], F32, kind="ExternalOutput")
    with tile.TileContext(nc) as tc:
        _body(tc, x_d, m_d, mask_d, wi_d, bi_d, wm_d, bm_d, wg_d, bg_d, o_d)
    nc.finalize()
    _NC_CACHE = nc
    return nc


def _in_maps(inputs, memory, mask, Wi, bi, Wm, bm, Wg, bg):
    maps = []
    for b in range(B):
        maps.append(
            {
                "x": np.ascontiguousarray(inputs[b], dtype=np.float32),
                "m": np.ascontiguousarray(memory[b], dtype=np.float32),
                "mask": np.ascontiguousarray(mask[b], dtype=np.int32),
                "Wi": np.ascontiguousarray(Wi, dtype=np.float32),
                "bi": np.ascontiguousarray(bi, dtype=np.float32),
                "Wm": np.ascontiguousarray(Wm, dtype=np.float32),
                "bm": np.ascontiguousarray(bm, dtype=np.float32),
                "Wg": np.ascontiguousarray(Wg, dtype=np.float32),
                "bg": np.ascontiguousarray(bg, dtype=np.float32),
            }
        )
    return maps


def run_spmd(inputs, memory, mask, Wi, bi, Wm, bm, Wg, bg, **spmd_kwargs):
    """Run the kernel across 8 cores; returns the BassKernelResults."""
    nc = _build_nc()
    maps = _in_maps(
        np.asarray(inputs), np.asarray(memory), np.asarray(mask),
        np.asarray(Wi), np.asarray(bi), np.asarray(Wm), np.asarray(bm),
        np.asarray(Wg), np.asarray(bg),
    )
    return run_bass_kernel_spmd(nc, maps, list(range(B)), **spmd_kwargs)


def kernel(inputs, memory, mask, Wi, bi, Wm, bm, Wg, bg):
    res = run_spmd(inputs, memory, mask, Wi, bi, Wm, bm, Wg, bg)
    out = np.stack([res.results[b]["out"] for b in range(B)], axis=0)
    return out.astype(np.float32)


# revision 10
# speedup vs baseline: 1.0843x; 1.0843x over previous
"""Trainium2 Bass kernel for nn_DotAttention (B=8 data-parallel over 8 cores).

Per core (one batch element), bf16 with one fp8 DoubleRow stage. v4:
all x.T/m.T transposes + projections run in the preamble, pipelined
through the then-free scores PSUM ring so the PE stays dense (full
p-state, fp32 transposes issue at ~110ns); preamble relus run on the
then-idle ACT engine.  The exp window carries only scores/U/uT/4 gates,
so the 32-exp stream never starves and has no table switches.  The tail
pipelines the remaining gates (PSUM ring reuse), 4-chunk sigmoids (bias
tied to the last exp so the scheduler cannot hoist them past it), the
gate*res mults, and per-quad output DMAs on two queues.

  x.T/m.T   : fp32 PE transposes; PSUM->SBUF copy casts to bf16 (DVE)
  xp/mp     : W.T @ {x,m}.T per 512 cols; bias+relu on ACT (preamble)
  S.T       : mp.T(:,jtile) @ xp.T, bf16, K=96
  e8        : exp(S.T*scale + maskbias) -> fp8e4 on ACT (table 0 only)
  U[jx,151] : fp8 DoubleRow vs [m|1]; denominator col 150; stride 171
  normalize : reciprocal_approx_fast + per-partition tensor_scalar -> bf16
  gate      : res.T chunks stationary, Wg moving; logits copied to SBUF
DMA: sync m0..m3+mask+Wg+bg, scalar(ACT) x0..x3, gpsimd Wi/bi/Wm/bm +
fp8 m casts.  PSUM: scores/preamble-pT/tail-gates share the "big" ring
(2x2 banks), U 3 banks, small shared bank for pp/uT/in-window gates.
"""

import contextlib
import math

import numpy as np

import concourse.bass as bass
import concourse.mybir as mybir
import concourse.tile as tile
from concourse import bacc
from concourse.bass_utils import run_bass_kernel_spmd
from concourse.masks import make_identity

F32 = mybir.dt.float32
F16 = mybir.dt.bfloat16
F8 = mybir.dt.float8e4
I32 = mybir.dt.int32
DR = mybir.MatmulPerfMode.DoubleRow

B = 8
JX = 2048
JM = 2048
D = 150
H = 96
G = 300
NJT = 16
NCH = 16
HALF = 1024
NSUB = HALF // 512
SCALE = 1.0 / math.sqrt(float(H))
NEG_BIG = 1.0e30


def _body(tc, x_d, m_d, mask_d, wi_d, bi_d, wm_d, bm_d, wg_d, bg_d, o_d):
    nc = tc.nc
    Relu = mybir.ActivationFunctionType.Relu
    Exp = mybir.ActivationFunctionType.Exp
    Sigmoid = mybir.ActivationFunctionType.Sigmoid
    MUL = mybir.AluOpType.mult
    SUB = mybir.AluOpType.subtract

    with contextlib.ExitStack() as ctx:
        const = ctx.enter_context(tc.tile_pool(name="const", bufs=1))
        work = ctx.enter_context(tc.tile_pool(name="work", bufs=2))
        epool = ctx.enter_context(tc.tile_pool(name="epool", bufs=3))
        psb = ctx.enter_context(tc.tile_pool(name="psb", bufs=2, space="PSUM"))
        pu = ctx.enter_context(tc.tile_pool(name="pu", bufs=1, space="PSUM"))

        ident16 = const.tile([128, 128], F16)
        make_identity(nc, ident16)
        ident32s = const.tile([NJT, NJT], F32)
        make_identity(nc, ident32s)
        ident32 = const.tile([128, 128], F32)
        make_identity(nc, ident32)

        # ---- input DMAs ---------------------------------------------------
        x_nat = const.tile([128, NCH, D], F32)
        m_nat = const.tile([128, NJT, D], F32)
        x_re = x_d.rearrange("(n p) d -> p n d", p=128)
        m_re = m_d.rearrange("(n p) d -> p n d", p=128)
        for g in range(4):
            gs4 = slice(g * 4, (g + 1) * 4)
            nc.sync.dma_start(out=m_nat[:, gs4, :], in_=m_re[:, gs4, :])
            nc.scalar.dma_start(out=x_nat[:, gs4, :], in_=x_re[:, gs4, :])
        mask_sb = const.tile([NJT, 128], I32)
        nc.sync.dma_start(out=mask_sb, in_=mask_d.rearrange("(n p) -> n p", p=128))

        wstage = const.tile([128, 2 * H], F32)
        wstage2 = const.tile([D - 128, 2 * H], F32)
        bi_sb = const.tile([H, 1], F32)
        bm_sb = const.tile([H, 1], F32)
        nc.gpsimd.dma_start(out=wstage[:, 0:H], in_=wi_d[0:128, :])
        nc.gpsimd.dma_start(out=wstage2[:, 0:H], in_=wi_d[128:D, :])
        nc.gpsimd.dma_start(out=bi_sb, in_=bi_d.rearrange("(n one) -> n one", one=1))
        nc.gpsimd.dma_start(out=wstage[:, H : 2 * H], in_=wm_d[0:128, :])
        nc.gpsimd.dma_start(out=wstage2[:, H : 2 * H], in_=wm_d[128:D, :])
        nc.gpsimd.dma_start(out=bm_sb, in_=bm_d.rearrange("(n one) -> n one", one=1))

        # ---- PE warmup while the first DMAs land -------------------------
        dummy = const.tile([1, 1], F32)
        jp = psb.tile([128, 128], F32, tag="sm", name="junk", bufs=1)
        for _ in range(12):
            nc.tensor.matmul(
                jp, ident16, ident16, start=True, stop=True,
                skip_group_check=True)
        nc.vector.tensor_copy(out=dummy, in_=jp[0:1, 0:1])

        # ---- weight casts (vector, tiny) ---------------------------------
        wi16a = const.tile([128, H], F16)
        nc.vector.tensor_copy(out=wi16a, in_=wstage[:, 0:H])
        wi16b = const.tile([D - 128, H], F16)
        nc.vector.tensor_copy(out=wi16b, in_=wstage2[:, 0:H])
        wm16a = const.tile([128, H], F16)
        nc.vector.tensor_copy(out=wm16a, in_=wstage[:, H : 2 * H])
        wm16b = const.tile([D - 128, H], F16)
        nc.vector.tensor_copy(out=wm16b, in_=wstage2[:, H : 2 * H])

        # ---- fp8 m (+ones col), 2-chunk units on gpsimd ------------------
        mt8 = const.tile([128, NJT, 176], F8)
        nc.gpsimd.memset(mt8[:, :, D:176], 0.0)
        nc.gpsimd.memset(mt8[:, :, 150:151], 1.0)
        for u in range(8):
            u2 = slice(u * 2, u * 2 + 2)
            nc.gpsimd.tensor_copy(out=mt8[:, u2, 0:D], in_=m_nat[:, u2, :])

        # ---- mask -> additive exp bias [128, NJT] ------------------------
        maskf = const.tile([NJT, 128], F32)
        nc.vector.tensor_copy(out=maskf, in_=mask_sb)
        nc.vector.tensor_scalar(
            out=maskf, in0=maskf, scalar1=1.0, scalar2=NEG_BIG,
            op0=SUB, op1=MUL)
        mb_ps = psb.tile([128, NJT], F32, tag="sm", name="mbps", bufs=1)
        nc.tensor.transpose(mb_ps, maskf, ident32s)
        maskbias = const.tile([128, NJT], F32)
        nc.vector.tensor_copy(out=maskbias, in_=mb_ps)

        # ---- transposed bf16 layouts --------------------------------------
        xT16a = const.tile([128, JX], F16)
        mT16a = const.tile([128, JM], F16)
        mT16b = const.tile([D - 128, JM], F16)
        # merged tail: x.T tail rows 0..21, U.T tail rows 32..53, ones row 64
        rtail = const.tile([65, JX], F16)
        nc.vector.memset(rtail, 0.0)
        nc.vector.memset(rtail[64:65, :], 1.0)

        xpT16 = const.tile([H, JX], F16)
        mpT16 = const.tile([H, JM], F16)

        def t_piece(which, p):
            # 2-chunk (256-col) fp32 transpose through the "big" ring
            src = x_nat if which == "x" else m_nat
            dstA = xT16a if which == "x" else mT16a
            dstB = rtail if which == "x" else mT16b
            pT = psb.tile([128, 2, 256], F32, tag="big", name="pT")
            for i in range(2):
                c = p * 2 + i
                nc.tensor.transpose(pT[:, i, 0:128], src[:, c, 0:128], ident32)
                nc.tensor.transpose(
                    pT[0 : D - 128, i, 128:256], src[:, c, 128:D], ident32)
            ss = slice(p * 256, (p + 1) * 256)
            nc.vector.tensor_copy(out=dstA[:, ss], in_=pT[:, :, 0:128])
            nc.vector.tensor_copy(
                out=dstB[0 : D - 128, ss], in_=pT[0 : D - 128, :, 128:256])

        def proj_sub(which, sub):
            # 512-col projection; bias+relu on the (idle) ACT engine
            if which == "x":
                wa, wb, b_sb, srcA, srcB, dst = (
                    wi16a, wi16b, bi_sb, xT16a, rtail, xpT16)
            else:
                wa, wb, b_sb, srcA, srcB, dst = (
                    wm16a, wm16b, bm_sb, mT16a, mT16b, mpT16)
            ss = slice(sub * 512, (sub + 1) * 512)
            pp = psb.tile([H, 512], F32, tag="sm", name="pp", bufs=1)
            nc.tensor.matmul(
                pp, wa, srcA[:, ss],
                start=True, stop=False, skip_group_check=True)
            nc.tensor.matmul(
                pp, wb, srcB[0 : D - 128, ss],
                start=False, stop=True, skip_group_check=True)
            nc.scalar.activation(
                out=dst[:, ss], in_=pp, func=Relu, bias=b_sb, scale=1.0)

        # ---- Wg/bg staged f32; cast on vector late in the preamble -------
        wg16a = const.tile([128, G], F16, tag="wg16a")
        wg16c = const.tile([128, G], F16, tag="wg16c")
        wgtail = const.tile([65, G], F16, tag="wgtail")
        nc.gpsimd.memset(wgtail, 0.0)
        wg_stages = []
        for sl, (g0, g1), w, r0 in ((0, (0, 128), wg16a, 0),
                                    (1, (128, 150), wgtail, 0),
                                    (2, (150, 278), wg16c, 0),
                                    (3, (278, 300), wgtail, 32)):
            wst = const.tile([g1 - g0, G], F32, tag=f"wgst_{sl}", name=f"wgst{sl}")
            nc.sync.dma_start(out=wst, in_=wg_d[g0:g1, :])
            wg_stages.append((wst, w, r0, g1 - g0))
        bgst = const.tile([1, G], F32, tag="bgst")
        nc.sync.dma_start(out=bgst, in_=bg_d.rearrange("(one n) -> one n", one=1))

        def cast_wg():
            for wst, w, r0, rows in wg_stages:
                nc.vector.tensor_copy(out=w[r0 : r0 + rows, :], in_=wst)
            nc.vector.tensor_copy(out=wgtail[64:65, :], in_=bgst)

        # ---- attention state ----------------------------------------------
        U16n = const.tile([128, NCH, 160], F16)
        nc.vector.memset(U16n[:, :, 150:160], 0.0)
        rcp_all = const.tile([128, NCH], F32)
        uT16a = const.tile([128, JX], F16)
        glog = const.tile([128, NCH, G], F32)
        gate16 = const.tile([128, NCH, G], F16)
        o_re = o_d.rearrange("(n p) k -> p n k", p=128)

        def ut_group(g):
            pA = psb.tile([128, 2, 256], F16, tag="sm", name="pUA", bufs=1)
            for i in range(2):
                c = g * 2 + i
                nc.tensor.transpose(
                    pA[:, i, 0:128], U16n[:, c, 0:128], ident16)
                nc.tensor.transpose(
                    pA[0 : D - 128, i, 128:256], U16n[:, c, 128:D], ident16)
            gcols = slice(g * 256, (g + 1) * 256)
            nc.vector.tensor_copy(out=uT16a[:, gcols], in_=pA[:, :, 0:128])
            nc.vector.tensor_copy(
                out=rtail[32 : 32 + D - 128, gcols],
                in_=pA[0 : D - 128, :, 128:256])

        def gate_chunk(c, tag="sm"):
            cs = slice(c * 128, (c + 1) * 128)
            gp = psb.tile([128, G], F32, tag=tag, name="gp",
                          bufs=1 if tag == "sm" else 2)
            for gi, (lhs, w) in enumerate((
                (xT16a[:, cs], wg16a), (uT16a[:, cs], wg16c),
                (rtail[:, cs], wgtail))):
                nc.tensor.matmul(
                    gp, lhs, w,
                    start=(gi == 0), stop=(gi == 2), skip_group_check=True)
            nc.vector.tensor_copy(out=glog[:, c, :], in_=gp)

        def norm_chunk(c, Up, h):
            nc.vector.tensor_scalar(
                out=U16n[:, c, 0:D], in0=Up[:, c - h * 8, 0:D],
                scalar1=rcp_all[:, c : c + 1],
                scalar2=None, op0=MUL)

        def emit_scores(h, j):
            sp = psb.tile([128, HALF], F32, tag="big", name="sp")
            for sx in range(NSUB):
                ss = slice(h * HALF + sx * 512, h * HALF + (sx + 1) * 512)
                nc.tensor.matmul(
                    sp[:, sx * 512 : (sx + 1) * 512],
                    mpT16[:, j * 128 : (j + 1) * 128], xpT16[:, ss],
                    start=True, stop=True, skip_group_check=True)
            return sp

        # ---- preamble: ALL transposes + projections ----------------------
        for which in ("x", "m"):
            for sub in range(4):
                t_piece(which, 2 * sub)
                t_piece(which, 2 * sub + 1)
                proj_sub(which, sub)
        cast_wg()
        sps = [emit_scores(0, 0), emit_scores(0, 1)]

        # ---- attention main loop ------------------------------------------
        Ups = [None, None]
        for h in range(2):
            Up = pu.tile([128, 8, 171], F32, tag="U", name="Up")
            Ups[h] = Up
            e_cur = epool.tile([128, 2, HALF], F8, tag="e8", name="e8")
            for t in range(NJT // 2):
                for s in range(2):
                    j = 2 * t + s
                    nc.scalar.activation(
                        out=e_cur[:, s, :], in_=sps[s], func=Exp,
                        bias=maskbias[:, j : j + 1], scale=SCALE)
                if t < NJT // 2 - 1:
                    sps = [emit_scores(h, 2 * t + 2),
                           emit_scores(h, 2 * t + 3)]
                elif h == 0:
                    sps = [emit_scores(1, 0), emit_scores(1, 1)]
                for c in range(8):
                    nc.tensor.matmul(
                        Up[:, c, 0:151],
                        e_cur[:, :, c * 128 : (c + 1) * 128],
                        mt8[:, 2 * t : 2 * t + 2, 0:151],
                        start=(t == 0), stop=(t == NJT // 2 - 1),
                        perf_mode=DR, skip_group_check=True)
                if h == 1:
                    if t == 0:
                        den = work.tile([128, 8], F32, tag="den")
                        nc.vector.tensor_copy(out=den, in_=Ups[0][:, :, 150])
                        nc.vector.reciprocal_approx_fast(
                            out=rcp_all[:, 0:8], in_=den)
                        for c in range(8):
                            norm_chunk(c, Ups[0], 0)
                        ut_group(0)
                    elif t == 1:
                        gate_chunk(0)
                    elif t == 2:
                        ut_group(1)
                    elif t == 3:
                        gate_chunk(1)
                    elif t == 4:
                        ut_group(2)
                    elif t == 5:
                        gate_chunk(2)
                    elif t == 6:
                        ut_group(3)
                    elif t == 7:
                        gate_chunk(3)
                if t < NJT // 2 - 1:
                    e_cur = epool.tile([128, 2, HALF], F8, tag="e8", name="e8")

        # ---- tail ---------------------------------------------------------
        # zero bias tied to the last exp: pins sigmoids after the exp stream
        zbias = const.tile([128, 1], F32)
        nc.vector.tensor_scalar(
            out=zbias, in0=e_cur[:, 1, 0:1], scalar1=0.0, scalar2=None,
            op0=MUL)

        den = work.tile([128, 8], F32, tag="den")
        nc.vector.tensor_copy(out=den, in_=Ups[1][:, :, 150])
        nc.vector.reciprocal_approx_fast(out=rcp_all[:, 8:16], in_=den)
        for c in range(8, 16):
            norm_chunk(c, Ups[1], 1)

        def sig_quad(q):
            c4 = slice(q * 4, q * 4 + 4)
            nc.scalar.activation(
                out=gate16[:, c4, :], in_=glog[:, c4, :], func=Sigmoid,
                bias=zbias, scale=1.0)

        def out_quad(q, dma_eng):
            c4 = slice(q * 4, q * 4 + 4)
            onat = work.tile([128, 4, G], F32, tag="onat", bufs=2)
            eng = nc.gpsimd if dma_eng is nc.sync else nc.vector
            eng.tensor_tensor(
                out=onat[:, :, 0:D], in0=gate16[:, c4, 0:D],
                in1=x_nat[:, c4, :], op=MUL)
            eng.tensor_tensor(
                out=onat[:, :, D:G], in0=gate16[:, c4, D:G],
                in1=U16n[:, c4, 0:D], op=MUL)
            dma_eng.dma_start(out=o_re[:, c4, :], in_=onat)

        gate_chunk(4, tag="big")
        gate_chunk(5, tag="big")
        gate_chunk(6, tag="big")
        gate_chunk(7, tag="big")
        sig_quad(0)
        out_quad(0, nc.sync)
        ut_group(4)
        gate_chunk(8, tag="big")
        gate_chunk(9, tag="big")
        sig_quad(1)
        out_quad(1, nc.gpsimd)
        ut_group(5)
        gate_chunk(10, tag="big")
        gate_chunk(11, tag="big")
        ut_group(6)
        gate_chunk(12, tag="big")
        gate_chunk(13, tag="big")
        sig_quad(2)
        out_quad(2, nc.sync)
        ut_group(7)
        gate_chunk(14, tag="big")
        gate_chunk(15, tag="big")
        sig_quad(3)
        out_quad(3, nc.gpsimd)


_NC_CACHE = None


def _build_nc():
    global _NC_CACHE
    if _NC_CACHE is not None:
        return _NC_CACHE
    nc = bacc.Bacc(None, target_bir_lowering=False, debug=False)
    x_d = nc.dram_tensor("x", [JX, D], F32, kind="ExternalInput")
    m_d = nc.dram_tensor("m", [JM, D], F32, kind="ExternalInput")
    mask_d = nc.dram_tensor("mask", [JM], I32, kind="ExternalInput")
    wi_d = nc.dram_tensor("Wi", [D, H], F32, kind="ExternalInput")
    bi_d = nc.dram_tensor("bi", [H], F32, kind="ExternalInput")
    wm_d = nc.dram_tensor("Wm", [D, H], F32, kind="ExternalInput")
    bm_d = nc.dram_tensor("bm", [H], F32, kind="ExternalInput")
    wg_d = nc.dram_tensor("Wg", [G, G], F32, kind="ExternalInput")
    bg_d = nc.dram_tensor("bg", [G], F32, kind="ExternalInput")
    o_d = nc.dram_tensor("out", [JX, G], F32, kind="ExternalOutput")
    with tile.TileContext(nc) as tc:
        _body(tc, x_d, m_d, mask_d, wi_d, bi_d, wm_d, bm_d, wg_d, bg_d, o_d)
    nc.finalize()
    _NC_CACHE = nc
    return nc


def _in_maps(inputs, memory, mask, Wi, bi, Wm, bm, Wg, bg):
    maps = []
    for b in range(B):
        maps.append(
            {
                "x": np.ascontiguousarray(inputs[b], dtype=np.float32),
                "m": np.ascontiguousarray(memory[b], dtype=np.float32),
                "mask": np.ascontiguousarray(mask[b], dtype=np.int32),
                "Wi": np.ascontiguousarray(Wi, dtype=np.float32),
                "bi": np.ascontiguousarray(bi, dtype=np.float32),
                "Wm": np.ascontiguousarray(Wm, dtype=np.float32),
                "bm": np.ascontiguousarray(bm, dtype=np.float32),
                "Wg": np.ascontiguousarray(Wg, dtype=np.float32),
                "bg": np.ascontiguousarray(bg, dtype=np.float32),
            }
        )
    return maps


def run_spmd(inputs, memory, mask, Wi, bi, Wm, bm, Wg, bg, **spmd_kwargs):
    """Run the kernel across 8 cores; returns the BassKernelResults."""
    nc = _build_nc()
    maps = _in_maps(
        np.asarray(inputs), np.asarray(memory), np.asarray(mask),
        np.asarray(Wi), np.asarray(bi), np.asarray(Wm), np.asarray(bm),
        np.asarray(Wg), np.asarray(bg),
    )
    return run_bass_kernel_spmd(nc, maps, list(range(B)), **spmd_kwargs)


def kernel(inputs, memory, mask, Wi, bi, Wm, bm, Wg, bg):
    res = run_spmd(inputs, memory, mask, Wi, bi, Wm, bm, Wg, bg)
    out = np.stack([res.results[b]["out"] for b in range(B)], axis=0)
    return out.astype(np.float32)


# revision 12
# speedup vs baseline: 1.3606x; 1.2548x over previous
"""Trainium2 Bass kernel for nn_DotAttention (B=8 data-parallel over 8 cores).

Per core (one batch element), bf16 with one fp8 DoubleRow stage. v4:
all x.T/m.T transposes + projections run in the preamble, pipelined
through the then-free scores PSUM ring so the PE stays dense (full
p-state, fp32 transposes issue at ~110ns); preamble relus run on the
then-idle ACT engine.  The exp window carries only scores/U/uT/4 gates,
so the 32-exp stream never starves and has no table switches.  The tail
pipelines the remaining gates (PSUM ring reuse), 4-chunk sigmoids (bias
tied to the last exp so the scheduler cannot hoist them past it), the
gate*res mults, and per-quad output DMAs on two queues.

  x.T/m.T   : fp32 PE transposes; PSUM->SBUF copy casts to bf16 (DVE)
  xp/mp     : W.T @ {x,m}.T per 512 cols; bias+relu on ACT (preamble)
  S.T       : mp.T(:,jtile) @ xp.T, bf16, K=96
  e8        : exp(S.T*scale + maskbias) -> fp8e4 on ACT (table 0 only)
  U[jx,151] : fp8 DoubleRow vs [m|1]; denominator col 150; stride 171
  normalize : reciprocal_approx_fast + per-partition tensor_scalar -> bf16
  gate      : res.T chunks stationary, Wg moving; logits copied to SBUF
DMA: sync m0..m3+mask+Wg+bg, scalar(ACT) x0..x3, gpsimd Wi/bi/Wm/bm +
fp8 m casts.  PSUM: scores/preamble-pT/tail-gates share the "big" ring
(2x2 banks), U 3 banks, small shared bank for pp/uT/in-window gates.
"""

import contextlib
import math

import numpy as np

import concourse.bass as bass
import concourse.mybir as mybir
import concourse.tile as tile
from concourse import bacc
from concourse.bass_utils import run_bass_kernel_spmd
from concourse.masks import make_identity

F32 = mybir.dt.float32
F16 = mybir.dt.bfloat16
F8 = mybir.dt.float8e4
I32 = mybir.dt.int32
DR = mybir.MatmulPerfMode.DoubleRow

B = 8
JX = 2048
JM = 2048
D = 150
H = 96
G = 300
NJT = 16
NCH = 16
HALF = 1024
NSUB = HALF // 512
SCALE = 1.0 / math.sqrt(float(H))
NEG_BIG = 1.0e30


def _body(tc, x_d, m_d, mask_d, wi_d, bi_d, wm_d, bm_d, wg_d, bg_d, o_d):
    nc = tc.nc
    Relu = mybir.ActivationFunctionType.Relu
    Exp = mybir.ActivationFunctionType.Exp
    Sigmoid = mybir.ActivationFunctionType.Sigmoid
    MUL = mybir.AluOpType.mult
    SUB = mybir.AluOpType.subtract

    with contextlib.ExitStack() as ctx:
        const = ctx.enter_context(tc.tile_pool(name="const", bufs=1))
        work = ctx.enter_context(tc.tile_pool(name="work", bufs=2))
        epool = ctx.enter_context(tc.tile_pool(name="epool", bufs=3))
        psb = ctx.enter_context(tc.tile_pool(name="psb", bufs=2, space="PSUM"))
        pu = ctx.enter_context(tc.tile_pool(name="pu", bufs=1, space="PSUM"))

        ident16 = const.tile([128, 128], F16)
        make_identity(nc, ident16)
        ident32s = const.tile([NJT, NJT], F32)
        make_identity(nc, ident32s)
        ident32 = const.tile([128, 128], F32)
        make_identity(nc, ident32)

        # ---- input DMAs (hardware DGE queues ONLY: sync + scalar; gpsimd
        # DMAs fall back to the slow software path) ------------------------
        x_nat = const.tile([128, NCH, D], F32)
        m_nat = const.tile([128, NJT, D], F32)
        x_re = x_d.rearrange("(n p) d -> p n d", p=128)
        m_re = m_d.rearrange("(n p) d -> p n d", p=128)
        wstage = const.tile([128, 2 * H], F32)
        wstage2 = const.tile([D - 128, 2 * H], F32)
        bi_sb = const.tile([H, 1], F32)
        bm_sb = const.tile([H, 1], F32)
        # scalar: x groups (needed first; ACT idle until the relus)
        for g in range(4):
            gs4 = slice(g * 4, (g + 1) * 4)
            nc.scalar.dma_start(out=x_nat[:, gs4, :], in_=x_re[:, gs4, :])
        # sync: weights (tiny), then m groups, mask, Wg late
        nc.sync.dma_start(out=wstage[:, 0:H], in_=wi_d[0:128, :])
        nc.sync.dma_start(out=wstage2[:, 0:H], in_=wi_d[128:D, :])
        nc.sync.dma_start(out=bi_sb, in_=bi_d.rearrange("(n one) -> n one", one=1))
        nc.sync.dma_start(out=wstage[:, H : 2 * H], in_=wm_d[0:128, :])
        nc.sync.dma_start(out=wstage2[:, H : 2 * H], in_=wm_d[128:D, :])
        nc.sync.dma_start(out=bm_sb, in_=bm_d.rearrange("(n one) -> n one", one=1))
        for g in range(4):
            gs4 = slice(g * 4, (g + 1) * 4)
            nc.sync.dma_start(out=m_nat[:, gs4, :], in_=m_re[:, gs4, :])
        mask_sb = const.tile([NJT, 128], I32)
        nc.sync.dma_start(out=mask_sb, in_=mask_d.rearrange("(n p) -> n p", p=128))

        # ---- PE warmup while the first DMAs land -------------------------
        dummy = const.tile([1, 1], F32)
        jp = psb.tile([128, 128], F32, tag="sm", name="junk", bufs=1)
        for _ in range(12):
            nc.tensor.matmul(
                jp, ident16, ident16, start=True, stop=True,
                skip_group_check=True)
        nc.vector.tensor_copy(out=dummy, in_=jp[0:1, 0:1])

        # ---- weight casts (vector, tiny) ---------------------------------
        wi16a = const.tile([128, H], F16)
        nc.vector.tensor_copy(out=wi16a, in_=wstage[:, 0:H])
        wi16b = const.tile([D - 128, H], F16)
        nc.vector.tensor_copy(out=wi16b, in_=wstage2[:, 0:H])
        wm16a = const.tile([128, H], F16)
        nc.vector.tensor_copy(out=wm16a, in_=wstage[:, H : 2 * H])
        wm16b = const.tile([D - 128, H], F16)
        nc.vector.tensor_copy(out=wm16b, in_=wstage2[:, H : 2 * H])

        # ---- fp8 m (+ones col), 2-chunk units on gpsimd ------------------
        mt8 = const.tile([128, NJT, 176], F8)
        nc.gpsimd.memset(mt8[:, :, D:176], 0.0)
        nc.gpsimd.memset(mt8[:, :, 150:151], 1.0)
        for u in range(8):
            u2 = slice(u * 2, u * 2 + 2)
            nc.gpsimd.tensor_copy(out=mt8[:, u2, 0:D], in_=m_nat[:, u2, :])

        # ---- mask -> additive exp bias [128, NJT] ------------------------
        maskf = const.tile([NJT, 128], F32)
        nc.vector.tensor_copy(out=maskf, in_=mask_sb)
        nc.vector.tensor_scalar(
            out=maskf, in0=maskf, scalar1=1.0, scalar2=NEG_BIG,
            op0=SUB, op1=MUL)
        mb_ps = psb.tile([128, NJT], F32, tag="sm", name="mbps", bufs=1)
        nc.tensor.transpose(mb_ps, maskf, ident32s)
        maskbias = const.tile([128, NJT], F32)
        nc.vector.tensor_copy(out=maskbias, in_=mb_ps)

        # ---- transposed bf16 layouts --------------------------------------
        xT16a = const.tile([128, JX], F16)
        mT16a = const.tile([128, JM], F16)
        mT16b = const.tile([D - 128, JM], F16)
        # merged tail: x.T tail rows 0..21, U.T tail rows 32..53, ones row 64
        rtail = const.tile([65, JX], F16)
        nc.vector.memset(rtail, 0.0)
        nc.vector.memset(rtail[64:65, :], 1.0)

        xpT16 = const.tile([H, JX], F16)
        mpT16 = const.tile([H, JM], F16)

        def t_piece(which, p):
            # 2-chunk (256-col) fp32 transpose through the "big" ring
            src = x_nat if which == "x" else m_nat
            dstA = xT16a if which == "x" else mT16a
            dstB = rtail if which == "x" else mT16b
            pT = psb.tile([128, 2, 256], F32, tag="big", name="pT")
            for i in range(2):
                c = p * 2 + i
                nc.tensor.transpose(pT[:, i, 0:128], src[:, c, 0:128], ident32)
                nc.tensor.transpose(
                    pT[0 : D - 128, i, 128:256], src[:, c, 128:D], ident32)
            ss = slice(p * 256, (p + 1) * 256)
            nc.vector.tensor_copy(out=dstA[:, ss], in_=pT[:, :, 0:128])
            nc.vector.tensor_copy(
                out=dstB[0 : D - 128, ss], in_=pT[0 : D - 128, :, 128:256])

        def proj_sub(which, sub):
            # 512-col projection; bias+relu on the (idle) ACT engine
            if which == "x":
                wa, wb, b_sb, srcA, srcB, dst = (
                    wi16a, wi16b, bi_sb, xT16a, rtail, xpT16)
            else:
                wa, wb, b_sb, srcA, srcB, dst = (
                    wm16a, wm16b, bm_sb, mT16a, mT16b, mpT16)
            ss = slice(sub * 512, (sub + 1) * 512)
            pp = psb.tile([H, 512], F32, tag="sm", name="pp", bufs=1)
            nc.tensor.matmul(
                pp, wa, srcA[:, ss],
                start=True, stop=False, skip_group_check=True)
            nc.tensor.matmul(
                pp, wb, srcB[0 : D - 128, ss],
                start=False, stop=True, skip_group_check=True)
            nc.scalar.activation(
                out=dst[:, ss], in_=pp, func=Relu, bias=b_sb, scale=1.0)

        # ---- Wg/bg staged f32; cast on vector late in the preamble -------
        wg16a = const.tile([128, G], F16, tag="wg16a")
        wg16c = const.tile([128, G], F16, tag="wg16c")
        wgtail = const.tile([65, G], F16, tag="wgtail")
        nc.gpsimd.memset(wgtail, 0.0)
        wg_stages = []
        for sl, (g0, g1), w, r0 in ((0, (0, 128), wg16a, 0),
                                    (1, (128, 150), wgtail, 0),
                                    (2, (150, 278), wg16c, 0),
                                    (3, (278, 300), wgtail, 32)):
            wst = const.tile([g1 - g0, G], F32, tag=f"wgst_{sl}", name=f"wgst{sl}")
            nc.sync.dma_start(out=wst, in_=wg_d[g0:g1, :])
            wg_stages.append((wst, w, r0, g1 - g0))
        bgst = const.tile([1, G], F32, tag="bgst")
        nc.sync.dma_start(out=bgst, in_=bg_d.rearrange("(one n) -> one n", one=1))

        def cast_wg():
            for wst, w, r0, rows in wg_stages:
                nc.vector.tensor_copy(out=w[r0 : r0 + rows, :], in_=wst)
            nc.vector.tensor_copy(out=wgtail[64:65, :], in_=bgst)

        # ---- attention state ----------------------------------------------
        U16n = const.tile([128, NCH, 160], F16)
        nc.vector.memset(U16n[:, :, 150:160], 0.0)
        rcp_all = const.tile([128, NCH], F32)
        uT16a = const.tile([128, JX], F16)
        glog = const.tile([128, NCH, G], F32)
        gate16 = const.tile([128, NCH, G], F16)
        o_re = o_d.rearrange("(n p) k -> p n k", p=128)

        def ut_group(g):
            pA = psb.tile([128, 2, 256], F16, tag="sm", name="pUA", bufs=1)
            for i in range(2):
                c = g * 2 + i
                nc.tensor.transpose(
                    pA[:, i, 0:128], U16n[:, c, 0:128], ident16)
                nc.tensor.transpose(
                    pA[0 : D - 128, i, 128:256], U16n[:, c, 128:D], ident16)
            gcols = slice(g * 256, (g + 1) * 256)
            nc.vector.tensor_copy(out=uT16a[:, gcols], in_=pA[:, :, 0:128])
            nc.vector.tensor_copy(
                out=rtail[32 : 32 + D - 128, gcols],
                in_=pA[0 : D - 128, :, 128:256])

        def gate_chunk(c, tag="sm"):
            cs = slice(c * 128, (c + 1) * 128)
            gp = psb.tile([128, G], F32, tag=tag, name="gp",
                          bufs=1 if tag == "sm" else 2)
            for gi, (lhs, w) in enumerate((
                (xT16a[:, cs], wg16a), (uT16a[:, cs], wg16c),
                (rtail[:, cs], wgtail))):
                nc.tensor.matmul(
                    gp, lhs, w,
                    start=(gi == 0), stop=(gi == 2), skip_group_check=True)
            nc.vector.tensor_copy(out=glog[:, c, :], in_=gp)

        def norm_chunk(c, Up, h):
            nc.vector.tensor_scalar(
                out=U16n[:, c, 0:D], in0=Up[:, c - h * 8, 0:D],
                scalar1=rcp_all[:, c : c + 1],
                scalar2=None, op0=MUL)

        def emit_scores(h, j):
            sp = psb.tile([128, HALF], F32, tag="big", name="sp")
            for sx in range(NSUB):
                ss = slice(h * HALF + sx * 512, h * HALF + (sx + 1) * 512)
                nc.tensor.matmul(
                    sp[:, sx * 512 : (sx + 1) * 512],
                    mpT16[:, j * 128 : (j + 1) * 128], xpT16[:, ss],
                    start=True, stop=True, skip_group_check=True)
            return sp

        # ---- preamble: ALL transposes + projections ----------------------
        for which in ("x", "m"):
            for sub in range(4):
                t_piece(which, 2 * sub)
                t_piece(which, 2 * sub + 1)
                proj_sub(which, sub)
        cast_wg()
        sps = [emit_scores(0, 0), emit_scores(0, 1)]

        # ---- attention main loop ------------------------------------------
        Ups = [None, None]
        for h in range(2):
            Up = pu.tile([128, 8, 171], F32, tag="U", name="Up")
            Ups[h] = Up
            e_cur = epool.tile([128, 2, HALF], F8, tag="e8", name="e8")
            for t in range(NJT // 2):
                for s in range(2):
                    j = 2 * t + s
                    nc.scalar.activation(
                        out=e_cur[:, s, :], in_=sps[s], func=Exp,
                        bias=maskbias[:, j : j + 1], scale=SCALE)
                if t < NJT // 2 - 1:
                    sps = [emit_scores(h, 2 * t + 2),
                           emit_scores(h, 2 * t + 3)]
                elif h == 0:
                    sps = [emit_scores(1, 0), emit_scores(1, 1)]
                for c in range(8):
                    nc.tensor.matmul(
                        Up[:, c, 0:151],
                        e_cur[:, :, c * 128 : (c + 1) * 128],
                        mt8[:, 2 * t : 2 * t + 2, 0:151],
                        start=(t == 0), stop=(t == NJT // 2 - 1),
                        perf_mode=DR, skip_group_check=True)
                if h == 1:
                    if t == 0:
                        den = work.tile([128, 8], F32, tag="den")
                        nc.vector.tensor_copy(out=den, in_=Ups[0][:, :, 150])
                        nc.vector.reciprocal_approx_fast(
                            out=rcp_all[:, 0:8], in_=den)
                        for c in range(8):
                            norm_chunk(c, Ups[0], 0)
                        ut_group(0)
                    elif t == 1:
                        gate_chunk(0)
                    elif t == 2:
                        ut_group(1)
                    elif t == 3:
                        gate_chunk(1)
                    elif t == 4:
                        ut_group(2)
                    elif t == 5:
                        gate_chunk(2)
                    elif t == 6:
                        ut_group(3)
                    elif t == 7:
                        gate_chunk(3)
                if t < NJT // 2 - 1:
                    e_cur = epool.tile([128, 2, HALF], F8, tag="e8", name="e8")

        # ---- tail ---------------------------------------------------------
        # zero bias tied to the last exp: pins sigmoids after the exp stream
        zbias = const.tile([128, 1], F32)
        nc.vector.tensor_scalar(
            out=zbias, in0=e_cur[:, 1, 0:1], scalar1=0.0, scalar2=None,
            op0=MUL)

        den = work.tile([128, 8], F32, tag="den")
        nc.vector.tensor_copy(out=den, in_=Ups[1][:, :, 150])
        nc.vector.reciprocal_approx_fast(out=rcp_all[:, 8:16], in_=den)
        for c in range(8, 16):
            norm_chunk(c, Ups[1], 1)

        def sig_quad(q):
            c4 = slice(q * 4, q * 4 + 4)
            nc.scalar.activation(
                out=gate16[:, c4, :], in_=glog[:, c4, :], func=Sigmoid,
                bias=zbias, scale=1.0)

        def out_quad(q, dma_eng):
            c4 = slice(q * 4, q * 4 + 4)
            onat = work.tile([128, 4, G], F32, tag="onat", bufs=2)
            eng = nc.gpsimd if dma_eng is nc.sync else nc.vector
            eng.tensor_tensor(
                out=onat[:, :, 0:D], in0=gate16[:, c4, 0:D],
                in1=x_nat[:, c4, :], op=MUL)
            eng.tensor_tensor(
                out=onat[:, :, D:G], in0=gate16[:, c4, D:G],
                in1=U16n[:, c4, 0:D], op=MUL)
            dma_eng.dma_start(out=o_re[:, c4, :], in_=onat)

        gate_chunk(4, tag="big")
        gate_chunk(5, tag="big")
        gate_chunk(6, tag="big")
        gate_chunk(7, tag="big")
        sig_quad(0)
        out_quad(0, nc.sync)
        ut_group(4)
        gate_chunk(8, tag="big")
        gate_chunk(9, tag="big")
        sig_quad(1)
        out_quad(1, nc.scalar)
        ut_group(5)
        gate_chunk(10, tag="big")
        gate_chunk(11, tag="big")
        ut_group(6)
        gate_chunk(12, tag="big")
        gate_chunk(13, tag="big")
        sig_quad(2)
        out_quad(2, nc.sync)
        ut_group(7)
        gate_chunk(14, tag="big")
        gate_chunk(15, tag="big")
        sig_quad(3)
        out_quad(3, nc.scalar)


_NC_CACHE = None


def _build_nc():
    global _NC_CACHE
    if _NC_CACHE is not None:
        return _NC_CACHE
    nc = bacc.Bacc(None, target_bir_lowering=False, debug=False)
    x_d = nc.dram_tensor("x", [JX, D], F32, kind="ExternalInput")
    m_d = nc.dram_tensor("m", [JM, D], F32, kind="ExternalInput")
    mask_d = nc.dram_tensor("mask", [JM], I32, kind="ExternalInput")
    wi_d = nc.dram_tensor("Wi", [D, H], F32, kind="ExternalInput")
    bi_d = nc.dram_tensor("bi", [H], F32, kind="ExternalInput")
    wm_d = nc.dram_tensor("Wm", [D, H], F32, kind="ExternalInput")
    bm_d = nc.dram_tensor("bm", [H], F32, kind="ExternalInput")
    wg_d = nc.dram_tensor("Wg", [G, G], F32, kind="ExternalInput")
    bg_d = nc.dram_tensor("bg", [G], F32, kind="ExternalInput")
    o_d = nc.dram_tensor("out", [JX, G], F32, kind="ExternalOutput")
    with tile.TileContext(nc) as tc:
        _body(tc, x_d, m_d, mask_d, wi_d, bi_d, wm_d, bm_d, wg_d, bg_d, o_d)
    nc.finalize()
    _NC_CACHE = nc
    return nc


def _in_maps(inputs, memory, mask, Wi, bi, Wm, bm, Wg, bg):
    maps = []
    for b in range(B):
        maps.append(
            {
                "x": np.ascontiguousarray(inputs[b], dtype=np.float32),
                "m": np.ascontiguousarray(memory[b], dtype=np.float32),
                "mask": np.ascontiguousarray(mask[b], dtype=np.int32),
                "Wi": np.ascontiguousarray(Wi, dtype=np.float32),
                "bi": np.ascontiguousarray(bi, dtype=np.float32),
                "Wm": np.ascontiguousarray(Wm, dtype=np.float32),
                "bm": np.ascontiguousarray(bm, dtype=np.float32),
                "Wg": np.ascontiguousarray(Wg, dtype=np.float32),
                "bg": np.ascontiguousarray(bg, dtype=np.float32),
            }
        )
    return maps


def run_spmd(inputs, memory, mask, Wi, bi, Wm, bm, Wg, bg, **spmd_kwargs):
    """Run the kernel across 8 cores; returns the BassKernelResults."""
    nc = _build_nc()
    maps = _in_maps(
        np.asarray(inputs), np.asarray(memory), np.asarray(mask),
        np.asarray(Wi), np.asarray(bi), np.asarray(Wm), np.asarray(bm),
        np.asarray(Wg), np.asarray(bg),
    )
    return run_bass_kernel_spmd(nc, maps, list(range(B)), **spmd_kwargs)


def kernel(inputs, memory, mask, Wi, bi, Wm, bm, Wg, bg):
    res = run_spmd(inputs, memory, mask, Wi, bi, Wm, bm, Wg, bg)
    out = np.stack([res.results[b]["out"] for b in range(B)], axis=0)
    return out.astype(np.float32)


# revision 13
# speedup vs baseline: 1.3989x; 1.0282x over previous
"""Trainium2 Bass kernel for nn_DotAttention (B=8 data-parallel over 8 cores).

Per core (one batch element), bf16 with one fp8 DoubleRow stage. v4:
all x.T/m.T transposes + projections run in the preamble, pipelined
through the then-free scores PSUM ring so the PE stays dense (full
p-state, fp32 transposes issue at ~110ns); preamble relus run on the
then-idle ACT engine.  The exp window carries only scores/U/uT/4 gates,
so the 32-exp stream never starves and has no table switches.  The tail
pipelines the remaining gates (PSUM ring reuse), 4-chunk sigmoids (bias
tied to the last exp so the scheduler cannot hoist them past it), the
gate*res mults, and per-quad output DMAs on two queues.

  x.T/m.T   : fp32 PE transposes; PSUM->SBUF copy casts to bf16 (DVE)
  xp/mp     : W.T @ {x,m}.T per 512 cols; bias+relu on ACT (preamble)
  S.T       : mp.T(:,jtile) @ xp.T, bf16, K=96
  e8        : exp(S.T*scale + maskbias) -> fp8e4 on ACT (table 0 only)
  U[jx,151] : fp8 DoubleRow vs [m|1]; denominator col 150; stride 171
  normalize : reciprocal_approx_fast + per-partition tensor_scalar -> bf16
  gate      : res.T chunks stationary, Wg moving; logits copied to SBUF
DMA: sync m0..m3+mask+Wg+bg, scalar(ACT) x0..x3, gpsimd Wi/bi/Wm/bm +
fp8 m casts.  PSUM: scores/preamble-pT/tail-gates share the "big" ring
(2x2 banks), U 3 banks, small shared bank for pp/uT/in-window gates.
"""

import contextlib
import math

import numpy as np

import concourse.bass as bass
import concourse.mybir as mybir
import concourse.tile as tile
from concourse import bacc
from concourse.bass_utils import run_bass_kernel_spmd
from concourse.masks import make_identity

F32 = mybir.dt.float32
F16 = mybir.dt.bfloat16
F8 = mybir.dt.float8e4
I32 = mybir.dt.int32
DR = mybir.MatmulPerfMode.DoubleRow

B = 8
JX = 2048
JM = 2048
D = 150
H = 96
G = 300
NJT = 16
NCH = 16
HALF = 1024
NSUB = HALF // 512
SCALE = 1.0 / math.sqrt(float(H))
NEG_BIG = 1.0e30


def _body(tc, x_d, m_d, mask_d, wi_d, bi_d, wm_d, bm_d, wg_d, bg_d, o_d):
    nc = tc.nc
    Relu = mybir.ActivationFunctionType.Relu
    Exp = mybir.ActivationFunctionType.Exp
    Sigmoid = mybir.ActivationFunctionType.Sigmoid
    MUL = mybir.AluOpType.mult
    SUB = mybir.AluOpType.subtract

    with contextlib.ExitStack() as ctx:
        const = ctx.enter_context(tc.tile_pool(name="const", bufs=1))
        work = ctx.enter_context(tc.tile_pool(name="work", bufs=2))
        epool = ctx.enter_context(tc.tile_pool(name="epool", bufs=3))
        psb = ctx.enter_context(tc.tile_pool(name="psb", bufs=2, space="PSUM"))
        pu = ctx.enter_context(tc.tile_pool(name="pu", bufs=1, space="PSUM"))

        ident16 = const.tile([128, 128], F16)
        make_identity(nc, ident16)
        ident32s = const.tile([NJT, NJT], F32)
        make_identity(nc, ident32s)
        ident32 = const.tile([128, 128], F32)
        make_identity(nc, ident32)

        # ---- input DMAs (hardware DGE queues ONLY: sync + scalar; gpsimd
        # DMAs fall back to the slow software path) ------------------------
        x_nat = const.tile([128, NCH, D], F32)
        m_nat = const.tile([128, NJT, D], F32)
        x_re = x_d.rearrange("(n p) d -> p n d", p=128)
        m_re = m_d.rearrange("(n p) d -> p n d", p=128)
        wstage = const.tile([128, 2 * H], F32)
        wstage2 = const.tile([D - 128, 2 * H], F32)
        bi_sb = const.tile([H, 1], F32)
        bm_sb = const.tile([H, 1], F32)
        # scalar: x groups (needed first; ACT idle until the relus)
        for g in range(4):
            gs4 = slice(g * 4, (g + 1) * 4)
            nc.scalar.dma_start(out=x_nat[:, gs4, :], in_=x_re[:, gs4, :])
        # sync: weights (tiny), then m groups, mask, Wg late
        nc.sync.dma_start(out=wstage[:, 0:H], in_=wi_d[0:128, :])
        nc.sync.dma_start(out=wstage2[:, 0:H], in_=wi_d[128:D, :])
        nc.sync.dma_start(out=bi_sb, in_=bi_d.rearrange("(n one) -> n one", one=1))
        nc.sync.dma_start(out=wstage[:, H : 2 * H], in_=wm_d[0:128, :])
        nc.sync.dma_start(out=wstage2[:, H : 2 * H], in_=wm_d[128:D, :])
        nc.sync.dma_start(out=bm_sb, in_=bm_d.rearrange("(n one) -> n one", one=1))
        for g in range(4):
            gs4 = slice(g * 4, (g + 1) * 4)
            nc.sync.dma_start(out=m_nat[:, gs4, :], in_=m_re[:, gs4, :])
        mask_sb = const.tile([NJT, 128], I32)
        nc.sync.dma_start(out=mask_sb, in_=mask_d.rearrange("(n p) -> n p", p=128))

        # ---- PE warmup while the first DMAs land -------------------------
        dummy = const.tile([1, 1], F32)
        jp = psb.tile([128, 128], F32, tag="sm", name="junk", bufs=1)
        for _ in range(18):
            nc.tensor.matmul(
                jp, ident16, ident16, start=True, stop=True,
                skip_group_check=True)
        nc.vector.tensor_copy(out=dummy, in_=jp[0:1, 0:1])

        # ---- weight casts (vector, tiny) ---------------------------------
        wi16a = const.tile([128, H], F16)
        nc.vector.tensor_copy(out=wi16a, in_=wstage[:, 0:H])
        wi16b = const.tile([D - 128, H], F16)
        nc.vector.tensor_copy(out=wi16b, in_=wstage2[:, 0:H])
        wm16a = const.tile([128, H], F16)
        nc.vector.tensor_copy(out=wm16a, in_=wstage[:, H : 2 * H])
        wm16b = const.tile([D - 128, H], F16)
        nc.vector.tensor_copy(out=wm16b, in_=wstage2[:, H : 2 * H])

        # ---- fp8 m (+ones col), 2-chunk units on gpsimd ------------------
        mt8 = const.tile([128, NJT, 176], F8)
        nc.gpsimd.memset(mt8[:, :, D:176], 0.0)
        nc.gpsimd.memset(mt8[:, :, 150:151], 1.0)
        for u in range(8):
            u2 = slice(u * 2, u * 2 + 2)
            nc.gpsimd.tensor_copy(out=mt8[:, u2, 0:D], in_=m_nat[:, u2, :])

        # ---- mask -> additive exp bias [128, NJT] ------------------------
        maskf = const.tile([NJT, 128], F32)
        nc.vector.tensor_copy(out=maskf, in_=mask_sb)
        nc.vector.tensor_scalar(
            out=maskf, in0=maskf, scalar1=1.0, scalar2=NEG_BIG,
            op0=SUB, op1=MUL)
        mb_ps = psb.tile([128, NJT], F32, tag="sm", name="mbps", bufs=1)
        nc.tensor.transpose(mb_ps, maskf, ident32s)
        maskbias = const.tile([128, NJT], F32)
        nc.vector.tensor_copy(out=maskbias, in_=mb_ps)

        # ---- transposed bf16 layouts --------------------------------------
        xT16a = const.tile([128, JX], F16)
        mT16a = const.tile([128, JM], F16)
        mT16b = const.tile([D - 128, JM], F16)
        # merged tail: x.T tail rows 0..21, U.T tail rows 32..53, ones row 64
        rtail = const.tile([65, JX], F16)
        nc.vector.memset(rtail, 0.0)
        nc.vector.memset(rtail[64:65, :], 1.0)

        xpT16 = const.tile([H, JX], F16)
        mpT16 = const.tile([H, JM], F16)

        piece_ring = ["big", "big", "sm"]
        piece_n = [0]

        def t_piece(which, p, ring=True):
            # 2-chunk (256-col) fp32 transpose; preamble pieces rotate a
            # 3-deep ring (big x2 + sm) so the PE never stalls on copies
            src = x_nat if which == "x" else m_nat
            dstA = xT16a if which == "x" else mT16a
            dstB = rtail if which == "x" else mT16b
            tag = piece_ring[piece_n[0] % 3] if ring else "sm"
            piece_n[0] += 1
            pT = psb.tile([128, 2, 256], F32, tag=tag, name="pT",
                          bufs=1 if tag == "sm" else 2)
            for i in range(2):
                c = p * 2 + i
                nc.tensor.transpose(pT[:, i, 0:128], src[:, c, 0:128], ident32)
                nc.tensor.transpose(
                    pT[0 : D - 128, i, 128:256], src[:, c, 128:D], ident32)
            ss = slice(p * 256, (p + 1) * 256)
            nc.vector.tensor_copy(out=dstA[:, ss], in_=pT[:, :, 0:128])
            nc.vector.tensor_copy(
                out=dstB[0 : D - 128, ss], in_=pT[0 : D - 128, :, 128:256])

        def proj_sub(which, sub, act=True, tag="U"):
            # 512-col projection; preamble relu on the (idle) ACT engine,
            # in-window relu on DVE; preamble pp via the not-yet-used U banks
            if which == "x":
                wa, wb, b_sb, srcA, srcB, dst = (
                    wi16a, wi16b, bi_sb, xT16a, rtail, xpT16)
            else:
                wa, wb, b_sb, srcA, srcB, dst = (
                    wm16a, wm16b, bm_sb, mT16a, mT16b, mpT16)
            ss = slice(sub * 512, (sub + 1) * 512)
            pool = pu if tag == "U" else psb
            pp = pool.tile([H, 512], F32, tag=tag, name="pp", bufs=1)
            nc.tensor.matmul(
                pp, wa, srcA[:, ss],
                start=True, stop=False, skip_group_check=True)
            nc.tensor.matmul(
                pp, wb, srcB[0 : D - 128, ss],
                start=False, stop=True, skip_group_check=True)
            if act:
                nc.scalar.activation(
                    out=dst[:, ss], in_=pp, func=Relu, bias=b_sb, scale=1.0)
            else:
                nc.vector.tensor_scalar(
                    out=dst[:, ss], in0=pp, scalar1=b_sb, scalar2=0.0,
                    op0=mybir.AluOpType.add, op1=mybir.AluOpType.max)

        # ---- Wg/bg staged f32; cast on vector late in the preamble -------
        wg16a = const.tile([128, G], F16, tag="wg16a")
        wg16c = const.tile([128, G], F16, tag="wg16c")
        wgtail = const.tile([65, G], F16, tag="wgtail")
        nc.gpsimd.memset(wgtail, 0.0)
        wg_stages = []
        for sl, (g0, g1), w, r0 in ((0, (0, 128), wg16a, 0),
                                    (1, (128, 150), wgtail, 0),
                                    (2, (150, 278), wg16c, 0),
                                    (3, (278, 300), wgtail, 32)):
            wst = const.tile([g1 - g0, G], F32, tag=f"wgst_{sl}", name=f"wgst{sl}")
            nc.sync.dma_start(out=wst, in_=wg_d[g0:g1, :])
            wg_stages.append((wst, w, r0, g1 - g0))
        bgst = const.tile([1, G], F32, tag="bgst")
        nc.sync.dma_start(out=bgst, in_=bg_d.rearrange("(one n) -> one n", one=1))

        def cast_wg():
            for wst, w, r0, rows in wg_stages:
                nc.vector.tensor_copy(out=w[r0 : r0 + rows, :], in_=wst)
            nc.vector.tensor_copy(out=wgtail[64:65, :], in_=bgst)

        # ---- attention state ----------------------------------------------
        U16n = const.tile([128, NCH, 160], F16)
        nc.vector.memset(U16n[:, :, 150:160], 0.0)
        rcp_all = const.tile([128, NCH], F32)
        uT16a = const.tile([128, JX], F16)
        glog = const.tile([128, NCH, G], F32)
        gate16 = const.tile([128, NCH, G], F16)
        o_re = o_d.rearrange("(n p) k -> p n k", p=128)

        def ut_group(g):
            pA = psb.tile([128, 2, 256], F16, tag="sm", name="pUA", bufs=1)
            for i in range(2):
                c = g * 2 + i
                nc.tensor.transpose(
                    pA[:, i, 0:128], U16n[:, c, 0:128], ident16)
                nc.tensor.transpose(
                    pA[0 : D - 128, i, 128:256], U16n[:, c, 128:D], ident16)
            gcols = slice(g * 256, (g + 1) * 256)
            nc.vector.tensor_copy(out=uT16a[:, gcols], in_=pA[:, :, 0:128])
            nc.vector.tensor_copy(
                out=rtail[32 : 32 + D - 128, gcols],
                in_=pA[0 : D - 128, :, 128:256])

        def gate_chunk(c, tag="sm"):
            cs = slice(c * 128, (c + 1) * 128)
            gp = psb.tile([128, G], F32, tag=tag, name="gp",
                          bufs=1 if tag == "sm" else 2)
            for gi, (lhs, w) in enumerate((
                (xT16a[:, cs], wg16a), (uT16a[:, cs], wg16c),
                (rtail[:, cs], wgtail))):
                nc.tensor.matmul(
                    gp, lhs, w,
                    start=(gi == 0), stop=(gi == 2), skip_group_check=True)
            nc.vector.tensor_copy(out=glog[:, c, :], in_=gp)

        def norm_chunk(c, Up, h):
            nc.vector.tensor_scalar(
                out=U16n[:, c, 0:D], in0=Up[:, c - h * 8, 0:D],
                scalar1=rcp_all[:, c : c + 1],
                scalar2=None, op0=MUL)

        def emit_scores(h, j):
            sp = psb.tile([128, HALF], F32, tag="big", name="sp")
            for sx in range(NSUB):
                ss = slice(h * HALF + sx * 512, h * HALF + (sx + 1) * 512)
                nc.tensor.matmul(
                    sp[:, sx * 512 : (sx + 1) * 512],
                    mpT16[:, j * 128 : (j + 1) * 128], xpT16[:, ss],
                    start=True, stop=True, skip_group_check=True)
            return sp

        # ---- preamble: x subs 0-3 + m subs 0-2 ---------------------------
        for sub in range(4):
            t_piece("x", 2 * sub)
            t_piece("x", 2 * sub + 1)
            proj_sub("x", sub)
        for sub in range(3):
            t_piece("m", 2 * sub)
            t_piece("m", 2 * sub + 1)
            proj_sub("m", sub)
        sps = [emit_scores(0, 0), emit_scores(0, 1)]

        # ---- attention main loop ------------------------------------------
        Ups = [None, None]
        for h in range(2):
            Up = pu.tile([128, 8, 171], F32, tag="U", name="Up")
            Ups[h] = Up
            e_cur = epool.tile([128, 2, HALF], F8, tag="e8", name="e8")
            for t in range(NJT // 2):
                for s in range(2):
                    j = 2 * t + s
                    nc.scalar.activation(
                        out=e_cur[:, s, :], in_=sps[s], func=Exp,
                        bias=maskbias[:, j : j + 1], scale=SCALE)
                if t < NJT // 2 - 1:
                    sps = [emit_scores(h, 2 * t + 2),
                           emit_scores(h, 2 * t + 3)]
                elif h == 0:
                    sps = [emit_scores(1, 0), emit_scores(1, 1)]
                for c in range(8):
                    nc.tensor.matmul(
                        Up[:, c, 0:151],
                        e_cur[:, :, c * 128 : (c + 1) * 128],
                        mt8[:, 2 * t : 2 * t + 2, 0:151],
                        start=(t == 0), stop=(t == NJT // 2 - 1),
                        perf_mode=DR, skip_group_check=True)
                if h == 0:
                    if t == 0:
                        t_piece("m", 6, ring=False)
                        t_piece("m", 7, ring=False)
                    elif t == 1:
                        proj_sub("m", 3, act=False, tag="sm")
                    elif t == 2:
                        cast_wg()
                if h == 1:
                    if t == 0:
                        den = work.tile([128, 8], F32, tag="den")
                        nc.vector.tensor_copy(out=den, in_=Ups[0][:, :, 150])
                        nc.vector.reciprocal_approx_fast(
                            out=rcp_all[:, 0:8], in_=den)
                        for c in range(8):
                            norm_chunk(c, Ups[0], 0)
                        ut_group(0)
                    elif t == 1:
                        gate_chunk(0)
                    elif t == 2:
                        ut_group(1)
                        gate_chunk(1)
                    elif t == 3:
                        gate_chunk(2)
                    elif t == 4:
                        ut_group(2)
                        gate_chunk(3)
                    elif t == 5:
                        gate_chunk(4)
                    elif t == 6:
                        ut_group(3)
                        gate_chunk(5)
                    elif t == 7:
                        gate_chunk(6)
                if t < NJT // 2 - 1:
                    e_cur = epool.tile([128, 2, HALF], F8, tag="e8", name="e8")

        # ---- tail ---------------------------------------------------------
        # zero bias tied to the last exp: pins sigmoids after the exp stream
        zbias = const.tile([128, 1], F32)
        nc.vector.tensor_scalar(
            out=zbias, in0=e_cur[:, 1, 0:1], scalar1=0.0, scalar2=None,
            op0=MUL)

        den = work.tile([128, 8], F32, tag="den")
        nc.vector.tensor_copy(out=den, in_=Ups[1][:, :, 150])
        nc.vector.reciprocal_approx_fast(out=rcp_all[:, 8:16], in_=den)
        for c in range(8, 16):
            norm_chunk(c, Ups[1], 1)

        def sig_quad(q):
            c4 = slice(q * 4, q * 4 + 4)
            nc.scalar.activation(
                out=gate16[:, c4, :], in_=glog[:, c4, :], func=Sigmoid,
                bias=zbias, scale=1.0)

        def out_quad(q, dma_eng):
            c4 = slice(q * 4, q * 4 + 4)
            onat = work.tile([128, 4, G], F32, tag="onat", bufs=2)
            eng = nc.gpsimd if dma_eng is nc.sync else nc.vector
            eng.tensor_tensor(
                out=onat[:, :, 0:D], in0=gate16[:, c4, 0:D],
                in1=x_nat[:, c4, :], op=MUL)
            eng.tensor_tensor(
                out=onat[:, :, D:G], in0=gate16[:, c4, D:G],
                in1=U16n[:, c4, 0:D], op=MUL)
            dma_eng.dma_start(out=o_re[:, c4, :], in_=onat)

        gate_chunk(7, tag="big")
        ut_group(4)
        ut_group(5)
        ut_group(6)
        ut_group(7)
        sig_quad(0)
        out_quad(0, nc.sync)
        gate_chunk(8, tag="big")
        gate_chunk(9, tag="big")
        sig_quad(1)
        out_quad(1, nc.scalar)
        gate_chunk(10, tag="big")
        gate_chunk(11, tag="big")
        gate_chunk(12, tag="big")
        gate_chunk(13, tag="big")
        sig_quad(2)
        out_quad(2, nc.sync)
        gate_chunk(14, tag="big")
        gate_chunk(15, tag="big")
        sig_quad(3)
        out_quad(3, nc.scalar)


_NC_CACHE = None


def _build_nc():
    global _NC_CACHE
    if _NC_CACHE is not None:
        return _NC_CACHE
    nc = bacc.Bacc(None, target_bir_lowering=False, debug=False)
    x_d = nc.dram_tensor("x", [JX, D], F32, kind="ExternalInput")
    m_d = nc.dram_tensor("m", [JM, D], F32, kind="ExternalInput")
    mask_d = nc.dram_tensor("mask", [JM], I32, kind="ExternalInput")
    wi_d = nc.dram_tensor("Wi", [D, H], F32, kind="ExternalInput")
    bi_d = nc.dram_tensor("bi", [H], F32, kind="ExternalInput")
    wm_d = nc.dram_tensor("Wm", [D, H], F32, kind="ExternalInput")
    bm_d = nc.dram_tensor("bm", [H], F32, kind="ExternalInput")
    wg_d = nc.dram_tensor("Wg", [G, G], F32, kind="ExternalInput")
    bg_d = nc.dram_tensor("bg", [G], F32, kind="ExternalInput")
    o_d = nc.dram_tensor("out", [JX, G], F32, kind="ExternalOutput")
    with tile.TileContext(nc) as tc:
        _body(tc, x_d, m_d, mask_d, wi_d, bi_d, wm_d, bm_d, wg_d, bg_d, o_d)
    nc.finalize()
    _NC_CACHE = nc
    return nc


def _in_maps(inputs, memory, mask, Wi, bi, Wm, bm, Wg, bg):
    maps = []
    for b in range(B):
        maps.append(
            {
                "x": np.ascontiguousarray(inputs[b], dtype=np.float32),
                "m": np.ascontiguousarray(memory[b], dtype=np.float32),
                "mask": np.ascontiguousarray(mask[b], dtype=np.int32),
                "Wi": np.ascontiguousarray(Wi, dtype=np.float32),
                "bi": np.ascontiguousarray(bi, dtype=np.float32),
                "Wm": np.ascontiguousarray(Wm, dtype=np.float32),
                "bm": np.ascontiguousarray(bm, dtype=np.float32),
                "Wg": np.ascontiguousarray(Wg, dtype=np.float32),
                "bg": np.ascontiguousarray(bg, dtype=np.float32),
            }
        )
    return maps


def run_spmd(inputs, memory, mask, Wi, bi, Wm, bm, Wg, bg, **spmd_kwargs):
    """Run the kernel across 8 cores; returns the BassKernelResults."""
    nc = _build_nc()
    maps = _in_maps(
        np.asarray(inputs), np.asarray(memory), np.asarray(mask),
        np.asarray(Wi), np.asarray(bi), np.asarray(Wm), np.asarray(bm),
        np.asarray(Wg), np.asarray(bg),
    )
    return run_bass_kernel_spmd(nc, maps, list(range(B)), **spmd_kwargs)


def kernel(inputs, memory, mask, Wi, bi, Wm, bm, Wg, bg):
    res = run_spmd(inputs, memory, mask, Wi, bi, Wm, bm, Wg, bg)
    out = np.stack([res.results[b]["out"] for b in range(B)], axis=0)
    return out.astype(np.float32)


# revision 14
# speedup vs baseline: 1.4659x; 1.0479x over previous
"""Trainium2 Bass kernel for nn_DotAttention (B=8 data-parallel over 8 cores).

Per core (one batch element), bf16 with one fp8 DoubleRow stage. v4:
all x.T/m.T transposes + projections run in the preamble, pipelined
through the then-free scores PSUM ring so the PE stays dense (full
p-state, fp32 transposes issue at ~110ns); preamble relus run on the
then-idle ACT engine.  The exp window carries only scores/U/uT/4 gates,
so the 32-exp stream never starves and has no table switches.  The tail
pipelines the remaining gates (PSUM ring reuse), 4-chunk sigmoids (bias
tied to the last exp so the scheduler cannot hoist them past it), the
gate*res mults, and per-quad output DMAs on two queues.

  x.T/m.T   : fp32 PE transposes; PSUM->SBUF copy casts to bf16 (DVE)
  xp/mp     : W.T @ {x,m}.T per 512 cols; bias+relu on ACT (preamble)
  S.T       : mp.T(:,jtile) @ xp.T, bf16, K=96
  e8        : exp(S.T*scale + maskbias) -> fp8e4 on ACT (table 0 only)
  U[jx,151] : fp8 DoubleRow vs [m|1]; denominator col 150; stride 171
  normalize : reciprocal_approx_fast + per-partition tensor_scalar -> bf16
  gate      : res.T chunks stationary, Wg moving; logits copied to SBUF
DMA: sync m0..m3+mask+Wg+bg, scalar(ACT) x0..x3, gpsimd Wi/bi/Wm/bm +
fp8 m casts.  PSUM: scores/preamble-pT/tail-gates share the "big" ring
(2x2 banks), U 3 banks, small shared bank for pp/uT/in-window gates.
"""

import contextlib
import math

import numpy as np

import concourse.bass as bass
import concourse.mybir as mybir
import concourse.tile as tile
from concourse import bacc
from concourse.bass_utils import run_bass_kernel_spmd
from concourse.masks import make_identity

F32 = mybir.dt.float32
F16 = mybir.dt.bfloat16
F8 = mybir.dt.float8e4
I32 = mybir.dt.int32
DR = mybir.MatmulPerfMode.DoubleRow

B = 8
JX = 2048
JM = 2048
D = 150
H = 96
G = 300
NJT = 16
NCH = 16
HALF = 1024
NSUB = HALF // 512
SCALE = 1.0 / math.sqrt(float(H))
NEG_BIG = 1.0e30


def _body(tc, x_d, m_d, mask_d, wi_d, bi_d, wm_d, bm_d, wg_d, bg_d, o_d):
    nc = tc.nc
    Relu = mybir.ActivationFunctionType.Relu
    Exp = mybir.ActivationFunctionType.Exp
    Sigmoid = mybir.ActivationFunctionType.Sigmoid
    MUL = mybir.AluOpType.mult
    SUB = mybir.AluOpType.subtract

    with contextlib.ExitStack() as ctx:
        const = ctx.enter_context(tc.tile_pool(name="const", bufs=1))
        work = ctx.enter_context(tc.tile_pool(name="work", bufs=2))
        epool = ctx.enter_context(tc.tile_pool(name="epool", bufs=3))
        psb = ctx.enter_context(tc.tile_pool(name="psb", bufs=2, space="PSUM"))
        pu = ctx.enter_context(tc.tile_pool(name="pu", bufs=1, space="PSUM"))

        ident16 = const.tile([128, 128], F16)
        make_identity(nc, ident16)
        ident32s = const.tile([NJT, NJT], F32)
        make_identity(nc, ident32s)
        ident32 = const.tile([128, 128], F32)
        make_identity(nc, ident32)

        # ---- input DMAs (hardware DGE queues ONLY: sync + scalar; gpsimd
        # DMAs fall back to the slow software path) ------------------------
        x_nat = const.tile([128, NCH, D], F32)
        m_nat = const.tile([128, NJT, D], F32)
        x_re = x_d.rearrange("(n p) d -> p n d", p=128)
        m_re = m_d.rearrange("(n p) d -> p n d", p=128)
        wstage = const.tile([128, 2 * H], F32)
        wstage2 = const.tile([D - 128, 2 * H], F32)
        bi_sb = const.tile([H, 1], F32)
        bm_sb = const.tile([H, 1], F32)
        # scalar: x groups (needed first; ACT idle until the relus)
        for g in range(4):
            gs4 = slice(g * 4, (g + 1) * 4)
            nc.scalar.dma_start(out=x_nat[:, gs4, :], in_=x_re[:, gs4, :])
        # sync: weights (tiny), then m groups, mask, Wg late
        nc.sync.dma_start(out=wstage[:, 0:H], in_=wi_d[0:128, :])
        nc.sync.dma_start(out=wstage2[:, 0:H], in_=wi_d[128:D, :])
        nc.sync.dma_start(out=bi_sb, in_=bi_d.rearrange("(n one) -> n one", one=1))
        nc.sync.dma_start(out=wstage[:, H : 2 * H], in_=wm_d[0:128, :])
        nc.sync.dma_start(out=wstage2[:, H : 2 * H], in_=wm_d[128:D, :])
        nc.sync.dma_start(out=bm_sb, in_=bm_d.rearrange("(n one) -> n one", one=1))
        mask_sb = const.tile([NJT, 128], I32)
        nc.sync.dma_start(out=mask_sb, in_=mask_d.rearrange("(n p) -> n p", p=128))
        for g in range(4):
            gs4 = slice(g * 4, (g + 1) * 4)
            nc.sync.dma_start(out=m_nat[:, gs4, :], in_=m_re[:, gs4, :])

        # ---- PE warmup while the first DMAs land -------------------------
        dummy = const.tile([1, 1], F32)
        jp = psb.tile([128, 128], F32, tag="sm", name="junk", bufs=1)
        for _ in range(18):
            nc.tensor.matmul(
                jp, ident16, ident16, start=True, stop=True,
                skip_group_check=True)
        nc.vector.tensor_copy(out=dummy, in_=jp[0:1, 0:1])

        # ---- weight casts (vector, tiny) ---------------------------------
        wi16a = const.tile([128, H], F16)
        nc.vector.tensor_copy(out=wi16a, in_=wstage[:, 0:H])
        wi16b = const.tile([D - 128, H], F16)
        nc.vector.tensor_copy(out=wi16b, in_=wstage2[:, 0:H])
        wm16a = const.tile([128, H], F16)
        nc.vector.tensor_copy(out=wm16a, in_=wstage[:, H : 2 * H])
        wm16b = const.tile([D - 128, H], F16)
        nc.vector.tensor_copy(out=wm16b, in_=wstage2[:, H : 2 * H])

        # ---- fp8 m (+ones col), 2-chunk units on gpsimd ------------------
        mt8 = const.tile([128, NJT, 176], F8)
        nc.gpsimd.memset(mt8[:, :, D:176], 0.0)
        nc.gpsimd.memset(mt8[:, :, 150:151], 1.0)
        for u in range(8):
            u2 = slice(u * 2, u * 2 + 2)
            nc.gpsimd.tensor_copy(out=mt8[:, u2, 0:D], in_=m_nat[:, u2, :])

        # ---- transposed bf16 layouts --------------------------------------
        xT16a = const.tile([128, JX], F16)
        mT16a = const.tile([128, JM], F16)
        mT16b = const.tile([D - 128, JM], F16)
        # merged tail: x.T tail rows 0..21, U.T tail rows 32..53, ones row 64
        rtail = const.tile([65, JX], F16)
        nc.vector.memset(rtail, 0.0)
        nc.vector.memset(rtail[64:65, :], 1.0)

        xpT16 = const.tile([H, JX], F16)
        mpT16 = const.tile([H, JM], F16)

        piece_ring = ["big", "big", "sm"]
        piece_n = [0]

        def t_piece(which, p, ring=True):
            # 2-chunk (256-col) fp32 transpose; preamble pieces rotate a
            # 3-deep ring (big x2 + sm) so the PE never stalls on copies
            src = x_nat if which == "x" else m_nat
            dstA = xT16a if which == "x" else mT16a
            dstB = rtail if which == "x" else mT16b
            tag = piece_ring[piece_n[0] % 3] if ring else "sm"
            piece_n[0] += 1
            pT = psb.tile([128, 2, 256], F32, tag=tag, name="pT",
                          bufs=1 if tag == "sm" else 2)
            for i in range(2):
                c = p * 2 + i
                nc.tensor.transpose(pT[:, i, 0:128], src[:, c, 0:128], ident32)
                nc.tensor.transpose(
                    pT[0 : D - 128, i, 128:256], src[:, c, 128:D], ident32)
            ss = slice(p * 256, (p + 1) * 256)
            nc.vector.tensor_copy(out=dstA[:, ss], in_=pT[:, :, 0:128])
            nc.vector.tensor_copy(
                out=dstB[0 : D - 128, ss], in_=pT[0 : D - 128, :, 128:256])

        def proj_sub(which, sub, act=True, tag="U"):
            # 512-col projection; preamble relu on the (idle) ACT engine,
            # in-window relu on DVE; preamble pp via the not-yet-used U banks
            if which == "x":
                wa, wb, b_sb, srcA, srcB, dst = (
                    wi16a, wi16b, bi_sb, xT16a, rtail, xpT16)
            else:
                wa, wb, b_sb, srcA, srcB, dst = (
                    wm16a, wm16b, bm_sb, mT16a, mT16b, mpT16)
            ss = slice(sub * 512, (sub + 1) * 512)
            pool = pu if tag == "U" else psb
            pp = pool.tile([H, 512], F32, tag=tag, name="pp", bufs=1)
            nc.tensor.matmul(
                pp, wa, srcA[:, ss],
                start=True, stop=False, skip_group_check=True)
            nc.tensor.matmul(
                pp, wb, srcB[0 : D - 128, ss],
                start=False, stop=True, skip_group_check=True)
            if act:
                nc.scalar.activation(
                    out=dst[:, ss], in_=pp, func=Relu, bias=b_sb, scale=1.0)
            else:
                nc.vector.tensor_scalar(
                    out=dst[:, ss], in0=pp, scalar1=b_sb, scalar2=0.0,
                    op0=mybir.AluOpType.add, op1=mybir.AluOpType.max)

        # ---- Wg/bg staged f32; cast on vector late in the preamble -------
        wg16a = const.tile([128, G], F16, tag="wg16a")
        wg16c = const.tile([128, G], F16, tag="wg16c")
        wgtail = const.tile([65, G], F16, tag="wgtail")
        nc.gpsimd.memset(wgtail, 0.0)
        wg_stages = []
        for sl, (g0, g1), w, r0 in ((0, (0, 128), wg16a, 0),
                                    (1, (128, 150), wgtail, 0),
                                    (2, (150, 278), wg16c, 0),
                                    (3, (278, 300), wgtail, 32)):
            wst = const.tile([g1 - g0, G], F32, tag=f"wgst_{sl}", name=f"wgst{sl}")
            nc.sync.dma_start(out=wst, in_=wg_d[g0:g1, :])
            wg_stages.append((wst, w, r0, g1 - g0))
        bgst = const.tile([1, G], F32, tag="bgst")
        nc.sync.dma_start(out=bgst, in_=bg_d.rearrange("(one n) -> one n", one=1))

        def cast_wg():
            for wst, w, r0, rows in wg_stages:
                nc.vector.tensor_copy(out=w[r0 : r0 + rows, :], in_=wst)
            nc.vector.tensor_copy(out=wgtail[64:65, :], in_=bgst)

        # ---- attention state ----------------------------------------------
        U16n = const.tile([128, NCH, 160], F16)
        nc.vector.memset(U16n[:, :, 150:160], 0.0)
        rcp_all = const.tile([128, NCH], F32)
        uT16a = const.tile([128, JX], F16)
        glog = const.tile([128, NCH, G], F32)
        gate16 = const.tile([128, NCH, G], F16)
        o_re = o_d.rearrange("(n p) k -> p n k", p=128)

        def ut_group(g):
            pA = psb.tile([128, 2, 256], F16, tag="sm", name="pUA", bufs=1)
            for i in range(2):
                c = g * 2 + i
                nc.tensor.transpose(
                    pA[:, i, 0:128], U16n[:, c, 0:128], ident16)
                nc.tensor.transpose(
                    pA[0 : D - 128, i, 128:256], U16n[:, c, 128:D], ident16)
            gcols = slice(g * 256, (g + 1) * 256)
            nc.vector.tensor_copy(out=uT16a[:, gcols], in_=pA[:, :, 0:128])
            nc.vector.tensor_copy(
                out=rtail[32 : 32 + D - 128, gcols],
                in_=pA[0 : D - 128, :, 128:256])

        def gate_chunk(c, tag="sm"):
            cs = slice(c * 128, (c + 1) * 128)
            gp = psb.tile([128, G], F32, tag=tag, name="gp",
                          bufs=1 if tag == "sm" else 2)
            for gi, (lhs, w) in enumerate((
                (xT16a[:, cs], wg16a), (uT16a[:, cs], wg16c),
                (rtail[:, cs], wgtail))):
                nc.tensor.matmul(
                    gp, lhs, w,
                    start=(gi == 0), stop=(gi == 2), skip_group_check=True)
            nc.vector.tensor_copy(out=glog[:, c, :], in_=gp)

        def norm_chunk(c, Up, h):
            nc.vector.tensor_scalar(
                out=U16n[:, c, 0:D], in0=Up[:, c - h * 8, 0:D],
                scalar1=rcp_all[:, c : c + 1],
                scalar2=None, op0=MUL)

        def emit_scores(h, j):
            sp = psb.tile([128, HALF], F32, tag="big", name="sp")
            for sx in range(NSUB):
                ss = slice(h * HALF + sx * 512, h * HALF + (sx + 1) * 512)
                nc.tensor.matmul(
                    sp[:, sx * 512 : (sx + 1) * 512],
                    mpT16[:, j * 128 : (j + 1) * 128], xpT16[:, ss],
                    start=True, stop=True, skip_group_check=True)
            return sp

        # ---- preamble: x subs 0-3 + m subs 0-2 ---------------------------
        for sub in range(4):
            t_piece("x", 2 * sub)
            t_piece("x", 2 * sub + 1)
            proj_sub("x", sub)
        for sub in range(3):
            t_piece("m", 2 * sub)
            t_piece("m", 2 * sub + 1)
            proj_sub("m", sub)
        # mask -> additive exp bias [128, NJT] (after the pieces so the
        # "sm" bank doesn't serialize the preamble behind the mask DMA)
        maskf = const.tile([NJT, 128], F32)
        nc.vector.tensor_copy(out=maskf, in_=mask_sb)
        nc.vector.tensor_scalar(
            out=maskf, in0=maskf, scalar1=1.0, scalar2=NEG_BIG,
            op0=SUB, op1=MUL)
        mb_ps = psb.tile([128, NJT], F32, tag="sm", name="mbps", bufs=1)
        nc.tensor.transpose(mb_ps, maskf, ident32s)
        maskbias = const.tile([128, NJT], F32)
        nc.vector.tensor_copy(out=maskbias, in_=mb_ps)

        sps = [emit_scores(0, 0), emit_scores(0, 1)]

        # ---- attention main loop ------------------------------------------
        Ups = [None, None]
        for h in range(2):
            Up = pu.tile([128, 8, 171], F32, tag="U", name="Up")
            Ups[h] = Up
            e_cur = epool.tile([128, 2, HALF], F8, tag="e8", name="e8")
            for t in range(NJT // 2):
                for s in range(2):
                    j = 2 * t + s
                    nc.scalar.activation(
                        out=e_cur[:, s, :], in_=sps[s], func=Exp,
                        bias=maskbias[:, j : j + 1], scale=SCALE)
                if t < NJT // 2 - 1:
                    sps = [emit_scores(h, 2 * t + 2),
                           emit_scores(h, 2 * t + 3)]
                elif h == 0:
                    sps = [emit_scores(1, 0), emit_scores(1, 1)]
                for c in range(8):
                    nc.tensor.matmul(
                        Up[:, c, 0:151],
                        e_cur[:, :, c * 128 : (c + 1) * 128],
                        mt8[:, 2 * t : 2 * t + 2, 0:151],
                        start=(t == 0), stop=(t == NJT // 2 - 1),
                        perf_mode=DR, skip_group_check=True)
                if h == 0:
                    if t == 0:
                        t_piece("m", 6, ring=False)
                        t_piece("m", 7, ring=False)
                    elif t == 1:
                        proj_sub("m", 3, act=False, tag="sm")
                    elif t == 2:
                        cast_wg()
                if h == 1:
                    if t == 0:
                        den = work.tile([128, 8], F32, tag="den")
                        nc.vector.tensor_copy(out=den, in_=Ups[0][:, :, 150])
                        nc.vector.reciprocal_approx_fast(
                            out=rcp_all[:, 0:8], in_=den)
                        for c in range(8):
                            norm_chunk(c, Ups[0], 0)
                        ut_group(0)
                    elif t == 1:
                        gate_chunk(0)
                    elif t == 2:
                        ut_group(1)
                        gate_chunk(1)
                    elif t == 3:
                        gate_chunk(2)
                    elif t == 4:
                        ut_group(2)
                        gate_chunk(3)
                    elif t == 5:
                        gate_chunk(4)
                    elif t == 6:
                        ut_group(3)
                        gate_chunk(5)
                    elif t == 7:
                        gate_chunk(6)
                if t < NJT // 2 - 1:
                    e_cur = epool.tile([128, 2, HALF], F8, tag="e8", name="e8")

        # ---- tail ---------------------------------------------------------
        # zero bias tied to the last exp: pins sigmoids after the exp stream
        zbias = const.tile([128, 1], F32)
        nc.vector.tensor_scalar(
            out=zbias, in0=e_cur[:, 1, 0:1], scalar1=0.0, scalar2=None,
            op0=MUL)

        den = work.tile([128, 8], F32, tag="den")
        nc.vector.tensor_copy(out=den, in_=Ups[1][:, :, 150])
        nc.vector.reciprocal_approx_fast(out=rcp_all[:, 8:16], in_=den)
        for c in range(8, 16):
            norm_chunk(c, Ups[1], 1)

        def sig_quad(q):
            c4 = slice(q * 4, q * 4 + 4)
            nc.scalar.activation(
                out=gate16[:, c4, :], in_=glog[:, c4, :], func=Sigmoid,
                bias=zbias, scale=1.0)

        def out_quad(q, dma_eng):
            c4 = slice(q * 4, q * 4 + 4)
            onat = work.tile([128, 4, G], F32, tag="onat", bufs=2)
            eng = nc.gpsimd if dma_eng is nc.sync else nc.vector
            eng.tensor_tensor(
                out=onat[:, :, 0:D], in0=gate16[:, c4, 0:D],
                in1=x_nat[:, c4, :], op=MUL)
            eng.tensor_tensor(
                out=onat[:, :, D:G], in0=gate16[:, c4, D:G],
                in1=U16n[:, c4, 0:D], op=MUL)
            dma_eng.dma_start(out=o_re[:, c4, :], in_=onat)

        gate_chunk(7, tag="big")
        jp2 = psb.tile([128, 128], F32, tag="sm", name="junk2", bufs=1)
        for _ in range(8):
            nc.tensor.matmul(
                jp2, ident16, ident16, start=True, stop=True,
                skip_group_check=True)
        nc.vector.tensor_copy(out=dummy, in_=jp2[0:1, 0:1])
        ut_group(4)
        ut_group(5)
        ut_group(6)
        ut_group(7)
        sig_quad(0)
        out_quad(0, nc.sync)
        gate_chunk(8, tag="big")
        gate_chunk(9, tag="big")
        sig_quad(1)
        out_quad(1, nc.scalar)
        gate_chunk(10, tag="big")
        gate_chunk(11, tag="big")
        gate_chunk(12, tag="big")
        gate_chunk(13, tag="big")
        sig_quad(2)
        out_quad(2, nc.sync)
        gate_chunk(14, tag="big")
        gate_chunk(15, tag="big")
        sig_quad(3)
        out_quad(3, nc.scalar)


_NC_CACHE = None


def _build_nc():
    global _NC_CACHE
    if _NC_CACHE is not None:
        return _NC_CACHE
    nc = bacc.Bacc(None, target_bir_lowering=False, debug=False)
    x_d = nc.dram_tensor("x", [JX, D], F32, kind="ExternalInput")
    m_d = nc.dram_tensor("m", [JM, D], F32, kind="ExternalInput")
    mask_d = nc.dram_tensor("mask", [JM], I32, kind="ExternalInput")
    wi_d = nc.dram_tensor("Wi", [D, H], F32, kind="ExternalInput")
    bi_d = nc.dram_tensor("bi", [H], F32, kind="ExternalInput")
    wm_d = nc.dram_tensor("Wm", [D, H], F32, kind="ExternalInput")
    bm_d = nc.dram_tensor("bm", [H], F32, kind="ExternalInput")
    wg_d = nc.dram_tensor("Wg", [G, G], F32, kind="ExternalInput")
    bg_d = nc.dram_tensor("bg", [G], F32, kind="ExternalInput")
    o_d = nc.dram_tensor("out", [JX, G], F32, kind="ExternalOutput")
    with tile.TileContext(nc) as tc:
        _body(tc, x_d, m_d, mask_d, wi_d, bi_d, wm_d, bm_d, wg_d, bg_d, o_d)
    nc.finalize()
    _NC_CACHE = nc
    return nc


def _in_maps(inputs, memory, mask, Wi, bi, Wm, bm, Wg, bg):
    maps = []
    for b in range(B):
        maps.append(
            {
                "x": np.ascontiguousarray(inputs[b], dtype=np.float32),
                "m": np.ascontiguousarray(memory[b], dtype=np.float32),
                "mask": np.ascontiguousarray(mask[b], dtype=np.int32),
                "Wi": np.ascontiguousarray(Wi, dtype=np.float32),
                "bi": np.ascontiguousarray(bi, dtype=np.float32),
                "Wm": np.ascontiguousarray(Wm, dtype=np.float32),
                "bm": np.ascontiguousarray(bm, dtype=np.float32),
                "Wg": np.ascontiguousarray(Wg, dtype=np.float32),
                "bg": np.ascontiguousarray(bg, dtype=np.float32),
            }
        )
    return maps


def run_spmd(inputs, memory, mask, Wi, bi, Wm, bm, Wg, bg, **spmd_kwargs):
    """Run the kernel across 8 cores; returns the BassKernelResults."""
    nc = _build_nc()
    maps = _in_maps(
        np.asarray(inputs), np.asarray(memory), np.asarray(mask),
        np.asarray(Wi), np.asarray(bi), np.asarray(Wm), np.asarray(bm),
        np.asarray(Wg), np.asarray(bg),
    )
    return run_bass_kernel_spmd(nc, maps, list(range(B)), **spmd_kwargs)


def kernel(inputs, memory, mask, Wi, bi, Wm, bm, Wg, bg):
    res = run_spmd(inputs, memory, mask, Wi, bi, Wm, bm, Wg, bg)
    out = np.stack([res.results[b]["out"] for b in range(B)], axis=0)
    return out.astype(np.float32)


# revision 17
# speedup vs baseline: 1.5097x; 1.0299x over previous
"""Trainium2 Bass kernel for nn_DotAttention (B=8 data-parallel over 8 cores).

Per core (one batch element), bf16 with one fp8 DoubleRow stage. v4:
all x.T/m.T transposes + projections run in the preamble, pipelined
through the then-free scores PSUM ring so the PE stays dense (full
p-state, fp32 transposes issue at ~110ns); preamble relus run on the
then-idle ACT engine.  The exp window carries only scores/U/uT/4 gates,
so the 32-exp stream never starves and has no table switches.  The tail
pipelines the remaining gates (PSUM ring reuse), 4-chunk sigmoids (bias
tied to the last exp so the scheduler cannot hoist them past it), the
gate*res mults, and per-quad output DMAs on two queues.

  x.T/m.T   : fp32 PE transposes; PSUM->SBUF copy casts to bf16 (DVE)
  xp/mp     : W.T @ {x,m}.T per 512 cols; bias+relu on ACT (preamble)
  S.T       : mp.T(:,jtile) @ xp.T, bf16, K=96
  e8        : exp(S.T*scale + maskbias) -> fp8e4 on ACT (table 0 only)
  U[jx,151] : fp8 DoubleRow vs [m|1]; denominator col 150; stride 171
  normalize : reciprocal_approx_fast + per-partition tensor_scalar -> bf16
  gate      : res.T chunks stationary, Wg moving; logits copied to SBUF
DMA: sync m0..m3+mask+Wg+bg, scalar(ACT) x0..x3, gpsimd Wi/bi/Wm/bm +
fp8 m casts.  PSUM: scores/preamble-pT/tail-gates share the "big" ring
(2x2 banks), U 3 banks, small shared bank for pp/uT/in-window gates.
"""

import contextlib
import math

import numpy as np

import concourse.bass as bass
import concourse.mybir as mybir
import concourse.tile as tile
from concourse import bacc
from concourse.bass_utils import run_bass_kernel_spmd
from concourse.masks import make_identity

F32 = mybir.dt.float32
F16 = mybir.dt.bfloat16
F8 = mybir.dt.float8e4
I32 = mybir.dt.int32
DR = mybir.MatmulPerfMode.DoubleRow

B = 8
JX = 2048
JM = 2048
D = 150
H = 96
G = 300
NJT = 16
NCH = 16
HALF = 1024
NSUB = HALF // 512
SCALE = 1.0 / math.sqrt(float(H))
NEG_BIG = 1.0e30


def _body(tc, x_d, m_d, mask_d, wi_d, bi_d, wm_d, bm_d, wg_d, bg_d, o_d):
    nc = tc.nc
    Relu = mybir.ActivationFunctionType.Relu
    Exp = mybir.ActivationFunctionType.Exp
    Sigmoid = mybir.ActivationFunctionType.Sigmoid
    MUL = mybir.AluOpType.mult
    SUB = mybir.AluOpType.subtract

    with contextlib.ExitStack() as ctx:
        const = ctx.enter_context(tc.tile_pool(name="const", bufs=1))
        work = ctx.enter_context(tc.tile_pool(name="work", bufs=2))
        epool = ctx.enter_context(tc.tile_pool(name="epool", bufs=3))
        psb = ctx.enter_context(tc.tile_pool(name="psb", bufs=2, space="PSUM"))
        pu = ctx.enter_context(tc.tile_pool(name="pu", bufs=1, space="PSUM"))

        ident16 = const.tile([128, 128], F16)
        make_identity(nc, ident16)
        ident32s = const.tile([NJT, NJT], F32)
        make_identity(nc, ident32s)
        ident32 = const.tile([128, 128], F32)
        make_identity(nc, ident32)

        # ---- input DMAs (hardware DGE queues ONLY: sync + scalar; gpsimd
        # DMAs fall back to the slow software path) ------------------------
        x_nat = const.tile([128, NCH, D], F32)
        m_nat = const.tile([128, NJT, D], F32)
        x_re = x_d.rearrange("(n p) d -> p n d", p=128)
        m_re = m_d.rearrange("(n p) d -> p n d", p=128)
        wstage = const.tile([128, 2 * H], F32)
        wstage2 = const.tile([D - 128, 2 * H], F32)
        bi_sb = const.tile([H, 1], F32)
        bm_sb = const.tile([H, 1], F32)
        # scalar: x groups (needed first; ACT idle until the relus)
        for g in range(4):
            gs4 = slice(g * 4, (g + 1) * 4)
            nc.scalar.dma_start(out=x_nat[:, gs4, :], in_=x_re[:, gs4, :])
        # sync: weights (tiny), then m groups, mask, Wg late
        nc.sync.dma_start(out=wstage[:, 0:H], in_=wi_d[0:128, :])
        nc.sync.dma_start(out=wstage2[:, 0:H], in_=wi_d[128:D, :])
        nc.sync.dma_start(out=bi_sb, in_=bi_d.rearrange("(n one) -> n one", one=1))
        nc.sync.dma_start(out=wstage[:, H : 2 * H], in_=wm_d[0:128, :])
        nc.sync.dma_start(out=wstage2[:, H : 2 * H], in_=wm_d[128:D, :])
        nc.sync.dma_start(out=bm_sb, in_=bm_d.rearrange("(n one) -> n one", one=1))
        mask_sb = const.tile([NJT, 128], I32)
        nc.sync.dma_start(out=mask_sb, in_=mask_d.rearrange("(n p) -> n p", p=128))
        for g in range(4):
            gs4 = slice(g * 4, (g + 1) * 4)
            nc.sync.dma_start(out=m_nat[:, gs4, :], in_=m_re[:, gs4, :])

        # ---- PE warmup while the first DMAs land -------------------------
        dummy = const.tile([1, 1], F32)
        jp = psb.tile([128, 128], F32, tag="sm", name="junk", bufs=1)
        for _ in range(18):
            nc.tensor.matmul(
                jp, ident16, ident16, start=True, stop=True,
                skip_group_check=True)
        nc.vector.tensor_copy(out=dummy, in_=jp[0:1, 0:1])

        # ---- weight casts (vector, tiny) ---------------------------------
        wi16a = const.tile([128, H], F16)
        nc.vector.tensor_copy(out=wi16a, in_=wstage[:, 0:H])
        wi16b = const.tile([D - 128, H], F16)
        nc.vector.tensor_copy(out=wi16b, in_=wstage2[:, 0:H])
        wm16a = const.tile([128, H], F16)
        nc.vector.tensor_copy(out=wm16a, in_=wstage[:, H : 2 * H])
        wm16b = const.tile([D - 128, H], F16)
        nc.vector.tensor_copy(out=wm16b, in_=wstage2[:, H : 2 * H])

        # ---- fp8 m (+ones col), 2-chunk units on gpsimd ------------------
        mt8 = const.tile([128, NJT, 176], F8)
        nc.gpsimd.memset(mt8[:, :, D:176], 0.0)
        nc.gpsimd.memset(mt8[:, :, 150:151], 1.0)
        for u in range(8):
            u2 = slice(u * 2, u * 2 + 2)
            nc.gpsimd.tensor_copy(out=mt8[:, u2, 0:D], in_=m_nat[:, u2, :])

        # ---- transposed bf16 layouts --------------------------------------
        xT16a = const.tile([128, JX], F16)
        mT16a = const.tile([128, JM], F16)
        mT16b = const.tile([D - 128, JM], F16)
        # merged tail: x.T tail rows 0..21, U.T tail rows 32..53, ones row 64
        rtail = const.tile([65, JX], F16)
        nc.vector.memset(rtail, 0.0)
        nc.vector.memset(rtail[64:65, :], 1.0)

        xpT16 = const.tile([H, JX], F16)
        mpT16 = const.tile([H, JM], F16)

        piece_ring = ["big", "big", "sm"]
        piece_n = [0]

        def t_piece(which, p, ring=True):
            # 2-chunk (256-col) fp32 transpose; preamble pieces rotate a
            # 3-deep ring (big x2 + sm) so the PE never stalls on copies
            src = x_nat if which == "x" else m_nat
            dstA = xT16a if which == "x" else mT16a
            dstB = rtail if which == "x" else mT16b
            tag = piece_ring[piece_n[0] % 3] if ring else "sm"
            piece_n[0] += 1
            pT = psb.tile([128, 2, 256], F32, tag=tag, name="pT",
                          bufs=1 if tag == "sm" else 2)
            for i in range(2):
                c = p * 2 + i
                nc.tensor.transpose(pT[:, i, 0:128], src[:, c, 0:128], ident32)
                nc.tensor.transpose(
                    pT[0 : D - 128, i, 128:256], src[:, c, 128:D], ident32)
            ss = slice(p * 256, (p + 1) * 256)
            nc.vector.tensor_copy(out=dstA[:, ss], in_=pT[:, :, 0:128])
            nc.vector.tensor_copy(
                out=dstB[0 : D - 128, ss], in_=pT[0 : D - 128, :, 128:256])

        def proj_sub(which, sub, act=True, tag="U"):
            # 512-col projection; preamble relu on the (idle) ACT engine,
            # in-window relu on DVE; preamble pp via the not-yet-used U banks
            if which == "x":
                wa, wb, b_sb, srcA, srcB, dst = (
                    wi16a, wi16b, bi_sb, xT16a, rtail, xpT16)
            else:
                wa, wb, b_sb, srcA, srcB, dst = (
                    wm16a, wm16b, bm_sb, mT16a, mT16b, mpT16)
            ss = slice(sub * 512, (sub + 1) * 512)
            pool = pu if tag == "U" else psb
            pp = pool.tile([H, 512], F32, tag=tag, name="pp", bufs=1)
            nc.tensor.matmul(
                pp, wa, srcA[:, ss],
                start=True, stop=False, skip_group_check=True)
            nc.tensor.matmul(
                pp, wb, srcB[0 : D - 128, ss],
                start=False, stop=True, skip_group_check=True)
            if act:
                nc.scalar.activation(
                    out=dst[:, ss], in_=pp, func=Relu, bias=b_sb, scale=1.0)
            else:
                nc.vector.tensor_scalar(
                    out=dst[:, ss], in0=pp, scalar1=b_sb, scalar2=0.0,
                    op0=mybir.AluOpType.add, op1=mybir.AluOpType.max)

        # ---- Wg/bg staged f32; cast on vector late in the preamble -------
        wg16a = const.tile([128, G], F16, tag="wg16a")
        wg16c = const.tile([128, G], F16, tag="wg16c")
        wgtail = const.tile([65, G], F16, tag="wgtail")
        nc.gpsimd.memset(wgtail, 0.0)
        wg_stages = []
        for sl, (g0, g1), w, r0 in ((0, (0, 128), wg16a, 0),
                                    (1, (128, 150), wgtail, 0),
                                    (2, (150, 278), wg16c, 0),
                                    (3, (278, 300), wgtail, 32)):
            wst = const.tile([g1 - g0, G], F32, tag=f"wgst_{sl}", name=f"wgst{sl}")
            nc.sync.dma_start(out=wst, in_=wg_d[g0:g1, :])
            wg_stages.append((wst, w, r0, g1 - g0))
        bgst = const.tile([1, G], F32, tag="bgst")
        nc.sync.dma_start(out=bgst, in_=bg_d.rearrange("(one n) -> one n", one=1))

        def cast_wg():
            for wst, w, r0, rows in wg_stages:
                nc.vector.tensor_copy(out=w[r0 : r0 + rows, :], in_=wst)
            nc.vector.tensor_copy(out=wgtail[64:65, :], in_=bgst)

        # ---- attention state ----------------------------------------------
        U16n = const.tile([128, NCH, 160], F16)
        nc.vector.memset(U16n[:, :, 150:160], 0.0)
        rcp_all = const.tile([128, NCH], F32)
        uT16a = const.tile([128, JX], F16)
        glog = const.tile([128, NCH, G], F32)
        gate16 = const.tile([128, NCH, G], F16)
        o_re = o_d.rearrange("(n p) k -> p n k", p=128)

        def ut_group(g):
            pA = psb.tile([128, 2, 256], F16, tag="sm", name="pUA", bufs=1)
            for i in range(2):
                c = g * 2 + i
                nc.tensor.transpose(
                    pA[:, i, 0:128], U16n[:, c, 0:128], ident16)
                nc.tensor.transpose(
                    pA[0 : D - 128, i, 128:256], U16n[:, c, 128:D], ident16)
            gcols = slice(g * 256, (g + 1) * 256)
            nc.vector.tensor_copy(out=uT16a[:, gcols], in_=pA[:, :, 0:128])
            nc.vector.tensor_copy(
                out=rtail[32 : 32 + D - 128, gcols],
                in_=pA[0 : D - 128, :, 128:256])

        def gate_chunk(c, tag="sm"):
            cs = slice(c * 128, (c + 1) * 128)
            gp = psb.tile([128, G], F32, tag=tag, name="gp",
                          bufs=1 if tag == "sm" else 2)
            for gi, (lhs, w) in enumerate((
                (xT16a[:, cs], wg16a), (uT16a[:, cs], wg16c),
                (rtail[:, cs], wgtail))):
                nc.tensor.matmul(
                    gp, lhs, w,
                    start=(gi == 0), stop=(gi == 2), skip_group_check=True)
            nc.vector.tensor_copy(out=glog[:, c, :], in_=gp)

        def norm_chunk(c, Up, h):
            nc.vector.tensor_scalar(
                out=U16n[:, c, 0:D], in0=Up[:, c - h * 8, 0:D],
                scalar1=rcp_all[:, c : c + 1],
                scalar2=None, op0=MUL)

        def emit_scores(h, j):
            sp = psb.tile([128, HALF], F32, tag="big", name="sp")
            for sx in range(NSUB):
                ss = slice(h * HALF + sx * 512, h * HALF + (sx + 1) * 512)
                nc.tensor.matmul(
                    sp[:, sx * 512 : (sx + 1) * 512],
                    mpT16[:, j * 128 : (j + 1) * 128], xpT16[:, ss],
                    start=True, stop=True, skip_group_check=True)
            return sp

        # ---- preamble: x subs 0-3 + m subs 0-2, projections lagging the
        # transpose pieces so the PE never waits on the DVE copies ----------
        t_piece("x", 0)
        t_piece("x", 1)
        t_piece("x", 2)
        proj_sub("x", 0)
        t_piece("x", 3)
        t_piece("x", 4)
        proj_sub("x", 1)
        t_piece("x", 5)
        t_piece("x", 6)
        proj_sub("x", 2)
        t_piece("x", 7)
        t_piece("m", 0)
        proj_sub("x", 3)
        t_piece("m", 1)
        t_piece("m", 2)
        proj_sub("m", 0)
        t_piece("m", 3)
        t_piece("m", 4)
        proj_sub("m", 1)
        t_piece("m", 5)
        proj_sub("m", 2)
        # mask -> additive exp bias [128, NJT] (after the pieces so the
        # "sm" bank doesn't serialize the preamble behind the mask DMA)
        maskf = const.tile([NJT, 128], F32)
        nc.vector.tensor_copy(out=maskf, in_=mask_sb)
        nc.vector.tensor_scalar(
            out=maskf, in0=maskf, scalar1=1.0, scalar2=NEG_BIG,
            op0=SUB, op1=MUL)
        mb_ps = psb.tile([128, NJT], F32, tag="sm", name="mbps", bufs=1)
        nc.tensor.transpose(mb_ps, maskf, ident32s)
        maskbias = const.tile([128, NJT], F32)
        nc.vector.tensor_copy(out=maskbias, in_=mb_ps)

        sps = [emit_scores(0, 0), emit_scores(0, 1)]

        # ---- attention main loop ------------------------------------------
        Ups = [None, None]
        for h in range(2):
            Up = pu.tile([128, 8, 171], F32, tag="U", name="Up")
            Ups[h] = Up
            e_cur = epool.tile([128, 2, HALF], F8, tag="e8", name="e8")
            for t in range(NJT // 2):
                for s in range(2):
                    j = 2 * t + s
                    nc.scalar.activation(
                        out=e_cur[:, s, :], in_=sps[s], func=Exp,
                        bias=maskbias[:, j : j + 1], scale=SCALE)
                if t < NJT // 2 - 1:
                    sps = [emit_scores(h, 2 * t + 2),
                           emit_scores(h, 2 * t + 3)]
                elif h == 0:
                    sps = [emit_scores(1, 0), emit_scores(1, 1)]
                for c in range(8):
                    nc.tensor.matmul(
                        Up[:, c, 0:151],
                        e_cur[:, :, c * 128 : (c + 1) * 128],
                        mt8[:, 2 * t : 2 * t + 2, 0:151],
                        start=(t == 0), stop=(t == NJT // 2 - 1),
                        perf_mode=DR, skip_group_check=True)
                if h == 0:
                    if t == 0:
                        t_piece("m", 6, ring=False)
                        t_piece("m", 7, ring=False)
                    elif t == 1:
                        proj_sub("m", 3, act=False, tag="sm")
                    elif t == 2:
                        cast_wg()
                if h == 1:
                    if t == 0:
                        den = work.tile([128, 8], F32, tag="den")
                        nc.vector.tensor_copy(out=den, in_=Ups[0][:, :, 150])
                        nc.vector.reciprocal_approx_fast(
                            out=rcp_all[:, 0:8], in_=den)
                        for c in range(8):
                            norm_chunk(c, Ups[0], 0)
                        ut_group(0)
                    elif t == 1:
                        gate_chunk(0)
                    elif t == 2:
                        ut_group(1)
                        gate_chunk(1)
                    elif t == 3:
                        gate_chunk(2)
                    elif t == 4:
                        ut_group(2)
                        gate_chunk(3)
                    elif t == 5:
                        gate_chunk(4)
                    elif t == 6:
                        ut_group(3)
                        gate_chunk(5)
                if t < NJT // 2 - 1:
                    e_cur = epool.tile([128, 2, HALF], F8, tag="e8", name="e8")

        # ---- tail ---------------------------------------------------------
        # zero bias tied to the last exp: pins sigmoids after the exp stream
        zbias = const.tile([128, 1], F32)
        nc.vector.tensor_scalar(
            out=zbias, in0=e_cur[:, 1, 0:1], scalar1=0.0, scalar2=None,
            op0=MUL)

        den = work.tile([128, 8], F32, tag="den")
        nc.vector.tensor_copy(out=den, in_=Ups[1][:, :, 150])
        nc.vector.reciprocal_approx_fast(out=rcp_all[:, 8:16], in_=den)
        for c in range(8, 16):
            norm_chunk(c, Ups[1], 1)

        def sig_quad(q):
            c4 = slice(q * 4, q * 4 + 4)
            nc.scalar.activation(
                out=gate16[:, c4, :], in_=glog[:, c4, :], func=Sigmoid,
                bias=zbias, scale=1.0)

        def gate_pair(ca):
            # two chunks into one PSUM "big" slot; the sigmoid reads PSUM
            # directly (no DVE copy on the tail critical path)
            gpp = psb.tile([128, 2, 512], F32, tag="big", name="gpp", bufs=2)
            for i, c in enumerate((ca, ca + 1)):
                cs = slice(c * 128, (c + 1) * 128)
                for gi, (lhs, w) in enumerate((
                    (xT16a[:, cs], wg16a), (uT16a[:, cs], wg16c),
                    (rtail[:, cs], wgtail))):
                    nc.tensor.matmul(
                        gpp[:, i, 0:G], lhs, w,
                        start=(gi == 0), stop=(gi == 2),
                        skip_group_check=True)
            return gpp

        def sig_pair_glog(ca):
            c2 = slice(ca, ca + 2)
            nc.scalar.activation(
                out=gate16[:, c2, :], in_=glog[:, c2, :], func=Sigmoid,
                bias=zbias, scale=1.0)

        def sig_pair_psum(ca, gpp):
            c2 = slice(ca, ca + 2)
            nc.scalar.activation(
                out=gate16[:, c2, :], in_=gpp[:, :, 0:G], func=Sigmoid,
                bias=zbias, scale=1.0)

        def out_quad(q, dma_eng):
            c4 = slice(q * 4, q * 4 + 4)
            onat = work.tile([128, 4, G], F32, tag="onat", bufs=2)
            eng = nc.gpsimd if dma_eng is nc.sync else nc.vector
            eng.tensor_tensor(
                out=onat[:, :, 0:D], in0=gate16[:, c4, 0:D],
                in1=x_nat[:, c4, :], op=MUL)
            eng.tensor_tensor(
                out=onat[:, :, D:G], in0=gate16[:, c4, D:G],
                in1=U16n[:, c4, 0:D], op=MUL)
            dma_eng.dma_start(out=o_re[:, c4, :], in_=onat)

        g67 = gate_pair(6)
        jp2 = psb.tile([128, 128], F32, tag="sm", name="junk2", bufs=1)
        for _ in range(8):
            nc.tensor.matmul(
                jp2, ident16, ident16, start=True, stop=True,
                skip_group_check=True)
        nc.vector.tensor_copy(out=dummy, in_=jp2[0:1, 0:1])
        sig_quad(0)
        sig_pair_glog(4)
        sig_pair_psum(6, g67)
        ut_group(4)
        g89 = gate_pair(8)
        sig_pair_psum(8, g89)
        out_quad(0, nc.sync)
        ut_group(5)
        gAB = gate_pair(10)
        sig_pair_psum(10, gAB)
        out_quad(1, nc.scalar)
        ut_group(6)
        gCD = gate_pair(12)
        sig_pair_psum(12, gCD)
        out_quad(2, nc.sync)
        ut_group(7)
        gEF = gate_pair(14)
        sig_pair_psum(14, gEF)
        out_quad(3, nc.scalar)


_NC_CACHE = None


def _build_nc():
    global _NC_CACHE
    if _NC_CACHE is not None:
        return _NC_CACHE
    nc = bacc.Bacc(None, target_bir_lowering=False, debug=False)
    x_d = nc.dram_tensor("x", [JX, D], F32, kind="ExternalInput")
    m_d = nc.dram_tensor("m", [JM, D], F32, kind="ExternalInput")
    mask_d = nc.dram_tensor("mask", [JM], I32, kind="ExternalInput")
    wi_d = nc.dram_tensor("Wi", [D, H], F32, kind="ExternalInput")
    bi_d = nc.dram_tensor("bi", [H], F32, kind="ExternalInput")
    wm_d = nc.dram_tensor("Wm", [D, H], F32, kind="ExternalInput")
    bm_d = nc.dram_tensor("bm", [H], F32, kind="ExternalInput")
    wg_d = nc.dram_tensor("Wg", [G, G], F32, kind="ExternalInput")
    bg_d = nc.dram_tensor("bg", [G], F32, kind="ExternalInput")
    o_d = nc.dram_tensor("out", [JX, G], F32, kind="ExternalOutput")
    with tile.TileContext(nc) as tc:
        _body(tc, x_d, m_d, mask_d, wi_d, bi_d, wm_d, bm_d, wg_d, bg_d, o_d)
    nc.finalize()
    _NC_CACHE = nc
    return nc


def _in_maps(inputs, memory, mask, Wi, bi, Wm, bm, Wg, bg):
    maps = []
    for b in range(B):
        maps.append(
            {
                "x": np.ascontiguousarray(inputs[b], dtype=np.float32),
                "m": np.ascontiguousarray(memory[b], dtype=np.float32),
                "mask": np.ascontiguousarray(mask[b], dtype=np.int32),
                "Wi": np.ascontiguousarray(Wi, dtype=np.float32),
                "bi": np.ascontiguousarray(bi, dtype=np.float32),
                "Wm": np.ascontiguousarray(Wm, dtype=np.float32),
                "bm": np.ascontiguousarray(bm, dtype=np.float32),
                "Wg": np.ascontiguousarray(Wg, dtype=np.float32),
                "bg": np.ascontiguousarray(bg, dtype=np.float32),
            }
        )
    return maps


def run_spmd(inputs, memory, mask, Wi, bi, Wm, bm, Wg, bg, **spmd_kwargs):
    """Run the kernel across 8 cores; returns the BassKernelResults."""
    nc = _build_nc()
    maps = _in_maps(
        np.asarray(inputs), np.asarray(memory), np.asarray(mask),
        np.asarray(Wi), np.asarray(bi), np.asarray(Wm), np.asarray(bm),
        np.asarray(Wg), np.asarray(bg),
    )
    return run_bass_kernel_spmd(nc, maps, list(range(B)), **spmd_kwargs)


def kernel(inputs, memory, mask, Wi, bi, Wm, bm, Wg, bg):
    res = run_spmd(inputs, memory, mask, Wi, bi, Wm, bm, Wg, bg)
    out = np.stack([res.results[b]["out"] for b in range(B)], axis=0)
    return out.astype(np.float32)


# revision 18
# speedup vs baseline: 1.5151x; 1.0035x over previous
"""Trainium2 Bass kernel for nn_DotAttention (B=8 data-parallel over 8 cores).

Per core (one batch element), bf16 with one fp8 DoubleRow stage. v4:
all x.T/m.T transposes + projections run in the preamble, pipelined
through the then-free scores PSUM ring so the PE stays dense (full
p-state, fp32 transposes issue at ~110ns); preamble relus run on the
then-idle ACT engine.  The exp window carries only scores/U/uT/4 gates,
so the 32-exp stream never starves and has no table switches.  The tail
pipelines the remaining gates (PSUM ring reuse), 4-chunk sigmoids (bias
tied to the last exp so the scheduler cannot hoist them past it), the
gate*res mults, and per-quad output DMAs on two queues.

  x.T/m.T   : fp32 PE transposes; PSUM->SBUF copy casts to bf16 (DVE)
  xp/mp     : W.T @ {x,m}.T per 512 cols; bias+relu on ACT (preamble)
  S.T       : mp.T(:,jtile) @ xp.T, bf16, K=96
  e8        : exp(S.T*scale + maskbias) -> fp8e4 on ACT (table 0 only)
  U[jx,151] : fp8 DoubleRow vs [m|1]; denominator col 150; stride 171
  normalize : reciprocal_approx_fast + per-partition tensor_scalar -> bf16
  gate      : res.T chunks stationary, Wg moving; logits copied to SBUF
DMA: sync m0..m3+mask+Wg+bg, scalar(ACT) x0..x3, gpsimd Wi/bi/Wm/bm +
fp8 m casts.  PSUM: scores/preamble-pT/tail-gates share the "big" ring
(2x2 banks), U 3 banks, small shared bank for pp/uT/in-window gates.
"""

import contextlib
import math

import numpy as np

import concourse.bass as bass
import concourse.mybir as mybir
import concourse.tile as tile
from concourse import bacc
from concourse.bass_utils import run_bass_kernel_spmd
from concourse.masks import make_identity

F32 = mybir.dt.float32
F16 = mybir.dt.bfloat16
F8 = mybir.dt.float8e4
I32 = mybir.dt.int32
DR = mybir.MatmulPerfMode.DoubleRow

B = 8
JX = 2048
JM = 2048
D = 150
H = 96
G = 300
NJT = 16
NCH = 16
HALF = 1024
NSUB = HALF // 512
SCALE = 1.0 / math.sqrt(float(H))
NEG_BIG = 1.0e30


def _body(tc, x_d, m_d, mask_d, wi_d, bi_d, wm_d, bm_d, wg_d, bg_d, o_d):
    nc = tc.nc
    Relu = mybir.ActivationFunctionType.Relu
    Exp = mybir.ActivationFunctionType.Exp
    Sigmoid = mybir.ActivationFunctionType.Sigmoid
    MUL = mybir.AluOpType.mult
    SUB = mybir.AluOpType.subtract

    with contextlib.ExitStack() as ctx:
        const = ctx.enter_context(tc.tile_pool(name="const", bufs=1))
        work = ctx.enter_context(tc.tile_pool(name="work", bufs=2))
        epool = ctx.enter_context(tc.tile_pool(name="epool", bufs=3))
        psb = ctx.enter_context(tc.tile_pool(name="psb", bufs=2, space="PSUM"))
        pu = ctx.enter_context(tc.tile_pool(name="pu", bufs=1, space="PSUM"))

        ident16 = const.tile([128, 128], F16)
        make_identity(nc, ident16)
        ident32s = const.tile([NJT, NJT], F32)
        make_identity(nc, ident32s)
        ident32 = const.tile([128, 128], F32)
        make_identity(nc, ident32)

        # ---- input DMAs (hardware DGE queues ONLY: sync + scalar; gpsimd
        # DMAs fall back to the slow software path) ------------------------
        x_nat = const.tile([128, NCH, D], F32)
        m_nat = const.tile([128, NJT, D], F32)
        x_re = x_d.rearrange("(n p) d -> p n d", p=128)
        m_re = m_d.rearrange("(n p) d -> p n d", p=128)
        wstage = const.tile([128, 2 * H], F32)
        wstage2 = const.tile([D - 128, 2 * H], F32)
        bi_sb = const.tile([H, 1], F32)
        bm_sb = const.tile([H, 1], F32)
        # scalar: x groups (needed first; ACT idle until the relus)
        for g in range(4):
            gs4 = slice(g * 4, (g + 1) * 4)
            nc.scalar.dma_start(out=x_nat[:, gs4, :], in_=x_re[:, gs4, :])
        # sync: mask + first m groups, weights, last m groups, Wg late
        mask_sb = const.tile([NJT, 128], I32)
        nc.sync.dma_start(out=mask_sb, in_=mask_d.rearrange("(n p) -> n p", p=128))
        for g in range(2):
            gs4 = slice(g * 4, (g + 1) * 4)
            nc.sync.dma_start(out=m_nat[:, gs4, :], in_=m_re[:, gs4, :])
        nc.sync.dma_start(out=wstage[:, 0:H], in_=wi_d[0:128, :])
        nc.sync.dma_start(out=wstage2[:, 0:H], in_=wi_d[128:D, :])
        nc.sync.dma_start(out=bi_sb, in_=bi_d.rearrange("(n one) -> n one", one=1))
        nc.sync.dma_start(out=wstage[:, H : 2 * H], in_=wm_d[0:128, :])
        nc.sync.dma_start(out=wstage2[:, H : 2 * H], in_=wm_d[128:D, :])
        nc.sync.dma_start(out=bm_sb, in_=bm_d.rearrange("(n one) -> n one", one=1))
        for g in range(2, 4):
            gs4 = slice(g * 4, (g + 1) * 4)
            nc.sync.dma_start(out=m_nat[:, gs4, :], in_=m_re[:, gs4, :])

        # ---- PE warmup while the first DMAs land -------------------------
        dummy = const.tile([1, 1], F32)
        jp = psb.tile([128, 128], F32, tag="sm", name="junk", bufs=1)
        for _ in range(18):
            nc.tensor.matmul(
                jp, ident16, ident16, start=True, stop=True,
                skip_group_check=True)
        nc.vector.tensor_copy(out=dummy, in_=jp[0:1, 0:1])

        # ---- weight casts (vector, tiny) ---------------------------------
        wi16a = const.tile([128, H], F16)
        nc.vector.tensor_copy(out=wi16a, in_=wstage[:, 0:H])
        wi16b = const.tile([D - 128, H], F16)
        nc.vector.tensor_copy(out=wi16b, in_=wstage2[:, 0:H])
        wm16a = const.tile([128, H], F16)
        nc.vector.tensor_copy(out=wm16a, in_=wstage[:, H : 2 * H])
        wm16b = const.tile([D - 128, H], F16)
        nc.vector.tensor_copy(out=wm16b, in_=wstage2[:, H : 2 * H])

        # ---- fp8 m (+ones col), 2-chunk units on gpsimd ------------------
        mt8 = const.tile([128, NJT, 176], F8)
        nc.gpsimd.memset(mt8[:, :, D:176], 0.0)
        nc.gpsimd.memset(mt8[:, :, 150:151], 1.0)
        for u in range(8):
            u2 = slice(u * 2, u * 2 + 2)
            nc.gpsimd.tensor_copy(out=mt8[:, u2, 0:D], in_=m_nat[:, u2, :])

        # ---- transposed bf16 layouts --------------------------------------
        xT16a = const.tile([128, JX], F16)
        mT16a = const.tile([128, JM], F16)
        mT16b = const.tile([D - 128, JM], F16)
        # merged tail: x.T tail rows 0..21, U.T tail rows 32..53, ones row 64
        rtail = const.tile([65, JX], F16)
        nc.vector.memset(rtail, 0.0)
        nc.vector.memset(rtail[64:65, :], 1.0)

        xpT16 = const.tile([H, JX], F16)
        mpT16 = const.tile([H, JM], F16)

        piece_ring = ["big", "big", "sm"]
        piece_n = [0]

        def t_piece(which, p, ring=True):
            # 2-chunk (256-col) fp32 transpose; preamble pieces rotate a
            # 3-deep ring (big x2 + sm) so the PE never stalls on copies
            src = x_nat if which == "x" else m_nat
            dstA = xT16a if which == "x" else mT16a
            dstB = rtail if which == "x" else mT16b
            tag = piece_ring[piece_n[0] % 3] if ring else "sm"
            piece_n[0] += 1
            pT = psb.tile([128, 2, 256], F32, tag=tag, name="pT",
                          bufs=1 if tag == "sm" else 2)
            for i in range(2):
                c = p * 2 + i
                nc.tensor.transpose(pT[:, i, 0:128], src[:, c, 0:128], ident32)
                nc.tensor.transpose(
                    pT[0 : D - 128, i, 128:256], src[:, c, 128:D], ident32)
            ss = slice(p * 256, (p + 1) * 256)
            nc.vector.tensor_copy(out=dstA[:, ss], in_=pT[:, :, 0:128])
            nc.vector.tensor_copy(
                out=dstB[0 : D - 128, ss], in_=pT[0 : D - 128, :, 128:256])
            if ring and piece_n[0] <= 10:
                # LDWEIGHTS fillers (no PSUM writes): keep the PE busy across
                # DMA-arrival gaps so the p-state ramp is never reset
                for _ in range(4):
                    nc.tensor.ldweights(ident16)

        def proj_sub(which, sub, act=True, tag="U"):
            # 512-col projection; preamble relu on the (idle) ACT engine,
            # in-window relu on DVE; preamble pp via the not-yet-used U banks
            if which == "x":
                wa, wb, b_sb, srcA, srcB, dst = (
                    wi16a, wi16b, bi_sb, xT16a, rtail, xpT16)
            else:
                wa, wb, b_sb, srcA, srcB, dst = (
                    wm16a, wm16b, bm_sb, mT16a, mT16b, mpT16)
            ss = slice(sub * 512, (sub + 1) * 512)
            pool = pu if tag == "U" else psb
            pp = pool.tile([H, 512], F32, tag=tag, name="pp", bufs=1)
            nc.tensor.matmul(
                pp, wa, srcA[:, ss],
                start=True, stop=False, skip_group_check=True)
            nc.tensor.matmul(
                pp, wb, srcB[0 : D - 128, ss],
                start=False, stop=True, skip_group_check=True)
            if act:
                nc.scalar.activation(
                    out=dst[:, ss], in_=pp, func=Relu, bias=b_sb, scale=1.0)
            else:
                nc.vector.tensor_scalar(
                    out=dst[:, ss], in0=pp, scalar1=b_sb, scalar2=0.0,
                    op0=mybir.AluOpType.add, op1=mybir.AluOpType.max)

        # ---- Wg/bg staged f32; cast on vector late in the preamble -------
        wg16a = const.tile([128, G], F16, tag="wg16a")
        wg16c = const.tile([128, G], F16, tag="wg16c")
        wgtail = const.tile([65, G], F16, tag="wgtail")
        nc.gpsimd.memset(wgtail, 0.0)
        wg_stages = []
        for sl, (g0, g1), w, r0 in ((0, (0, 128), wg16a, 0),
                                    (1, (128, 150), wgtail, 0),
                                    (2, (150, 278), wg16c, 0),
                                    (3, (278, 300), wgtail, 32)):
            wst = const.tile([g1 - g0, G], F32, tag=f"wgst_{sl}", name=f"wgst{sl}")
            nc.sync.dma_start(out=wst, in_=wg_d[g0:g1, :])
            wg_stages.append((wst, w, r0, g1 - g0))
        bgst = const.tile([1, G], F32, tag="bgst")
        nc.sync.dma_start(out=bgst, in_=bg_d.rearrange("(one n) -> one n", one=1))

        def cast_wg():
            for wst, w, r0, rows in wg_stages:
                nc.vector.tensor_copy(out=w[r0 : r0 + rows, :], in_=wst)
            nc.vector.tensor_copy(out=wgtail[64:65, :], in_=bgst)

        # ---- attention state ----------------------------------------------
        U16n = const.tile([128, NCH, 160], F16)
        nc.vector.memset(U16n[:, :, 150:160], 0.0)
        rcp_all = const.tile([128, NCH], F32)
        uT16a = const.tile([128, JX], F16)
        glog = const.tile([128, NCH, G], F32)
        gate16 = const.tile([128, NCH, G], F16)
        o_re = o_d.rearrange("(n p) k -> p n k", p=128)

        def ut_group(g):
            pA = psb.tile([128, 2, 256], F16, tag="sm", name="pUA", bufs=1)
            for i in range(2):
                c = g * 2 + i
                nc.tensor.transpose(
                    pA[:, i, 0:128], U16n[:, c, 0:128], ident16)
                nc.tensor.transpose(
                    pA[0 : D - 128, i, 128:256], U16n[:, c, 128:D], ident16)
            gcols = slice(g * 256, (g + 1) * 256)
            nc.vector.tensor_copy(out=uT16a[:, gcols], in_=pA[:, :, 0:128])
            nc.vector.tensor_copy(
                out=rtail[32 : 32 + D - 128, gcols],
                in_=pA[0 : D - 128, :, 128:256])

        def gate_chunk(c, tag="sm"):
            cs = slice(c * 128, (c + 1) * 128)
            gp = psb.tile([128, G], F32, tag=tag, name="gp",
                          bufs=1 if tag == "sm" else 2)
            for gi, (lhs, w) in enumerate((
                (xT16a[:, cs], wg16a), (uT16a[:, cs], wg16c),
                (rtail[:, cs], wgtail))):
                nc.tensor.matmul(
                    gp, lhs, w,
                    start=(gi == 0), stop=(gi == 2), skip_group_check=True)
            nc.vector.tensor_copy(out=glog[:, c, :], in_=gp)

        def norm_chunk(c, Up, h):
            nc.vector.tensor_scalar(
                out=U16n[:, c, 0:D], in0=Up[:, c - h * 8, 0:D],
                scalar1=rcp_all[:, c : c + 1],
                scalar2=None, op0=MUL)

        def emit_scores(h, j):
            sp = psb.tile([128, HALF], F32, tag="big", name="sp")
            for sx in range(NSUB):
                ss = slice(h * HALF + sx * 512, h * HALF + (sx + 1) * 512)
                nc.tensor.matmul(
                    sp[:, sx * 512 : (sx + 1) * 512],
                    mpT16[:, j * 128 : (j + 1) * 128], xpT16[:, ss],
                    start=True, stop=True, skip_group_check=True)
            return sp

        # ---- preamble: x subs 0-3 + m subs 0-2, projections lagging the
        # transpose pieces so the PE never waits on the DVE copies ----------
        t_piece("x", 0)
        t_piece("x", 1)
        t_piece("x", 2)
        proj_sub("x", 0)
        t_piece("x", 3)
        t_piece("x", 4)
        proj_sub("x", 1)
        t_piece("x", 5)
        t_piece("x", 6)
        proj_sub("x", 2)
        t_piece("x", 7)
        t_piece("m", 0)
        proj_sub("x", 3)
        t_piece("m", 1)
        t_piece("m", 2)
        proj_sub("m", 0)
        t_piece("m", 3)
        t_piece("m", 4)
        proj_sub("m", 1)
        t_piece("m", 5)
        proj_sub("m", 2)
        # mask -> additive exp bias [128, NJT] (after the pieces so the
        # "sm" bank doesn't serialize the preamble behind the mask DMA)
        maskf = const.tile([NJT, 128], F32)
        nc.vector.tensor_copy(out=maskf, in_=mask_sb)
        nc.vector.tensor_scalar(
            out=maskf, in0=maskf, scalar1=1.0, scalar2=NEG_BIG,
            op0=SUB, op1=MUL)
        mb_ps = psb.tile([128, NJT], F32, tag="sm", name="mbps", bufs=1)
        nc.tensor.transpose(mb_ps, maskf, ident32s)
        maskbias = const.tile([128, NJT], F32)
        nc.vector.tensor_copy(out=maskbias, in_=mb_ps)

        sps = [emit_scores(0, 0), emit_scores(0, 1)]

        # ---- attention main loop ------------------------------------------
        Ups = [None, None]
        for h in range(2):
            Up = pu.tile([128, 8, 171], F32, tag="U", name="Up")
            Ups[h] = Up
            e_cur = epool.tile([128, 2, HALF], F8, tag="e8", name="e8")
            for t in range(NJT // 2):
                for s in range(2):
                    j = 2 * t + s
                    nc.scalar.activation(
                        out=e_cur[:, s, :], in_=sps[s], func=Exp,
                        bias=maskbias[:, j : j + 1], scale=SCALE)
                if t < NJT // 2 - 1:
                    sps = [emit_scores(h, 2 * t + 2),
                           emit_scores(h, 2 * t + 3)]
                elif h == 0:
                    sps = [emit_scores(1, 0), emit_scores(1, 1)]
                for c in range(8):
                    nc.tensor.matmul(
                        Up[:, c, 0:151],
                        e_cur[:, :, c * 128 : (c + 1) * 128],
                        mt8[:, 2 * t : 2 * t + 2, 0:151],
                        start=(t == 0), stop=(t == NJT // 2 - 1),
                        perf_mode=DR, skip_group_check=True)
                if h == 0:
                    if t == 0:
                        t_piece("m", 6, ring=False)
                        t_piece("m", 7, ring=False)
                    elif t == 1:
                        proj_sub("m", 3, act=False, tag="sm")
                    elif t == 2:
                        cast_wg()
                if h == 1:
                    if t == 0:
                        den = work.tile([128, 8], F32, tag="den")
                        nc.vector.tensor_copy(out=den, in_=Ups[0][:, :, 150])
                        nc.vector.reciprocal_approx_fast(
                            out=rcp_all[:, 0:8], in_=den)
                        for c in range(8):
                            norm_chunk(c, Ups[0], 0)
                        ut_group(0)
                    elif t == 1:
                        gate_chunk(0)
                    elif t == 2:
                        ut_group(1)
                        gate_chunk(1)
                    elif t == 3:
                        gate_chunk(2)
                    elif t == 4:
                        ut_group(2)
                        gate_chunk(3)
                    elif t == 5:
                        gate_chunk(4)
                    elif t == 6:
                        ut_group(3)
                        gate_chunk(5)
                if t < NJT // 2 - 1:
                    e_cur = epool.tile([128, 2, HALF], F8, tag="e8", name="e8")

        # ---- tail ---------------------------------------------------------
        # zero bias tied to the last exp: pins sigmoids after the exp stream
        zbias = const.tile([128, 1], F32)
        nc.vector.tensor_scalar(
            out=zbias, in0=e_cur[:, 1, 0:1], scalar1=0.0, scalar2=None,
            op0=MUL)

        den = work.tile([128, 8], F32, tag="den")
        nc.vector.tensor_copy(out=den, in_=Ups[1][:, :, 150])
        nc.vector.reciprocal_approx_fast(out=rcp_all[:, 8:16], in_=den)
        for c in range(8, 16):
            norm_chunk(c, Ups[1], 1)

        def sig_quad(q):
            c4 = slice(q * 4, q * 4 + 4)
            nc.scalar.activation(
                out=gate16[:, c4, :], in_=glog[:, c4, :], func=Sigmoid,
                bias=zbias, scale=1.0)

        def gate_pair(ca):
            # two chunks into one PSUM "big" slot; the sigmoid reads PSUM
            # directly (no DVE copy on the tail critical path)
            gpp = psb.tile([128, 2, 512], F32, tag="big", name="gpp", bufs=2)
            for i, c in enumerate((ca, ca + 1)):
                cs = slice(c * 128, (c + 1) * 128)
                for gi, (lhs, w) in enumerate((
                    (xT16a[:, cs], wg16a), (uT16a[:, cs], wg16c),
                    (rtail[:, cs], wgtail))):
                    nc.tensor.matmul(
                        gpp[:, i, 0:G], lhs, w,
                        start=(gi == 0), stop=(gi == 2),
                        skip_group_check=True)
            return gpp

        def sig_pair_glog(ca):
            c2 = slice(ca, ca + 2)
            nc.scalar.activation(
                out=gate16[:, c2, :], in_=glog[:, c2, :], func=Sigmoid,
                bias=zbias, scale=1.0)

        def sig_pair_psum(ca, gpp):
            c2 = slice(ca, ca + 2)
            nc.scalar.activation(
                out=gate16[:, c2, :], in_=gpp[:, :, 0:G], func=Sigmoid,
                bias=zbias, scale=1.0)

        def out_quad(q, dma_eng):
            c4 = slice(q * 4, q * 4 + 4)
            onat = work.tile([128, 4, G], F32, tag="onat", bufs=2)
            eng = nc.gpsimd if dma_eng is nc.sync else nc.vector
            eng.tensor_tensor(
                out=onat[:, :, 0:D], in0=gate16[:, c4, 0:D],
                in1=x_nat[:, c4, :], op=MUL)
            eng.tensor_tensor(
                out=onat[:, :, D:G], in0=gate16[:, c4, D:G],
                in1=U16n[:, c4, 0:D], op=MUL)
            dma_eng.dma_start(out=o_re[:, c4, :], in_=onat)

        g67 = gate_pair(6)
        jp2 = psb.tile([128, 128], F32, tag="sm", name="junk2", bufs=1)
        for _ in range(8):
            nc.tensor.matmul(
                jp2, ident16, ident16, start=True, stop=True,
                skip_group_check=True)
        nc.vector.tensor_copy(out=dummy, in_=jp2[0:1, 0:1])
        sig_quad(0)
        sig_pair_glog(4)
        sig_pair_psum(6, g67)
        ut_group(4)
        g89 = gate_pair(8)
        sig_pair_psum(8, g89)
        out_quad(0, nc.sync)
        ut_group(5)
        gAB = gate_pair(10)
        sig_pair_psum(10, gAB)
        out_quad(1, nc.scalar)
        ut_group(6)
        gCD = gate_pair(12)
        sig_pair_psum(12, gCD)
        out_quad(2, nc.sync)
        ut_group(7)
        gEF = gate_pair(14)
        sig_pair_psum(14, gEF)
        out_quad(3, nc.scalar)


_NC_CACHE = None


def _build_nc():
    global _NC_CACHE
    if _NC_CACHE is not None:
        return _NC_CACHE
    nc = bacc.Bacc(None, target_bir_lowering=False, debug=False)
    x_d = nc.dram_tensor("x", [JX, D], F32, kind="ExternalInput")
    m_d = nc.dram_tensor("m", [JM, D], F32, kind="ExternalInput")
    mask_d = nc.dram_tensor("mask", [JM], I32, kind="ExternalInput")
    wi_d = nc.dram_tensor("Wi", [D, H], F32, kind="ExternalInput")
    bi_d = nc.dram_tensor("bi", [H], F32, kind="ExternalInput")
    wm_d = nc.dram_tensor("Wm", [D, H], F32, kind="ExternalInput")
    bm_d = nc.dram_tensor("bm", [H], F32, kind="ExternalInput")
    wg_d = nc.dram_tensor("Wg", [G, G], F32, kind="ExternalInput")
    bg_d = nc.dram_tensor("bg", [G], F32, kind="ExternalInput")
    o_d = nc.dram_tensor("out", [JX, G], F32, kind="ExternalOutput")
    with tile.TileContext(nc) as tc:
        _body(tc, x_d, m_d, mask_d, wi_d, bi_d, wm_d, bm_d, wg_d, bg_d, o_d)
    nc.finalize()
    _NC_CACHE = nc
    return nc


def _in_maps(inputs, memory, mask, Wi, bi, Wm, bm, Wg, bg):
    maps = []
    for b in range(B):
        maps.append(
            {
                "x": np.ascontiguousarray(inputs[b], dtype=np.float32),
                "m": np.ascontiguousarray(memory[b], dtype=np.float32),
                "mask": np.ascontiguousarray(mask[b], dtype=np.int32),
                "Wi": np.ascontiguousarray(Wi, dtype=np.float32),
                "bi": np.ascontiguousarray(bi, dtype=np.float32),
                "Wm": np.ascontiguousarray(Wm, dtype=np.float32),
                "bm": np.ascontiguousarray(bm, dtype=np.float32),
                "Wg": np.ascontiguousarray(Wg, dtype=np.float32),
                "bg": np.ascontiguousarray(bg, dtype=np.float32),
            }
        )
    return maps


def run_spmd(inputs, memory, mask, Wi, bi, Wm, bm, Wg, bg, **spmd_kwargs):
    """Run the kernel across 8 cores; returns the BassKernelResults."""
    nc = _build_nc()
    maps = _in_maps(
        np.asarray(inputs), np.asarray(memory), np.asarray(mask),
        np.asarray(Wi), np.asarray(bi), np.asarray(Wm), np.asarray(bm),
        np.asarray(Wg), np.asarray(bg),
    )
    return run_bass_kernel_spmd(nc, maps, list(range(B)), **spmd_kwargs)


def kernel(inputs, memory, mask, Wi, bi, Wm, bm, Wg, bg):
    res = run_spmd(inputs, memory, mask, Wi, bi, Wm, bm, Wg, bg)
    out = np.stack([res.results[b]["out"] for b in range(B)], axis=0)
    return out.astype(np.float32)
